# revision 8
# baseline (speedup 1.0000x reference)
"""Bipartite 3-layer GAT (user<->item) on 8 Trainium2 NeuronCores.

Strategy:
- Destination-range sharding: core k owns user-nodes and item-nodes
  [k*2500, (k+1)*2500) as edge destinations. All edges whose dst falls in
  that range are processed by core k (sorted by dst on host).
- Node phase is sharded by node rows; each core computes its slice of
  h = a @ W (plus per-head attention projections es/ed), then an
  AllGather replicates the (h|es) table so every core can gather rows
  for arbitrary source nodes.
- Edge phase: per 128-dst block, batched dma_gather of source rows
  (h|es) and of per-dst ed rows, softmax weights w = exp(leaky(es+ed))
  (the segment-max cancels mathematically and scores are small, so it
  is skipped), messages scaled by w, and a one-hot matmul segment-sum
  accumulated in PSUM. Softmax denominators accumulate in the same
  PSUM tile via a second matmul against w.
- Host does index preprocessing only (shard/sort/pad); all FLOPs and
  data movement run on device.
"""
import sys

for _p in ("/opt/trn_rl_repo",):
    if _p not in sys.path:
        sys.path.insert(0, _p)

import numpy as np

P = 128


class Cfg:
    def __init__(self, n=20000, e=320000, f_in=128, hid=64, heads=4, nl=3,
                 neg=0.2, ncores=8):
        self.n, self.e, self.f_in, self.hid = n, e, f_in, hid
        self.heads, self.nl, self.neg, self.ncores = heads, nl, neg, ncores
        self.nsr = n // ncores                    # raw nodes per core
        self.nb = (self.nsr + P - 1) // P         # dst blocks per core
        self.ns = self.nb * P                     # padded nodes per core
        self.d1 = heads * hid                     # 256
        self.tblw = self.d1 + 64                  # [hs d1 | es 4 | pad] 256B-mult
        self.tbl_rows = ncores * self.ns
        self.edw = 64                             # ed table width (256B rows)


def _wrap_idx(ix):
    """[T] int -> [128, T//16] int16 (16-partition wrap, replicated 8x)."""
    assert ix.shape[0] % 16 == 0
    w = ix.reshape(-1, 16).T.astype(np.int16)
    return np.ascontiguousarray(np.tile(w, (8, 1)))


def _prep_dir(edge, cfg):
    """Shard+sort one edge direction by dst range. Returns
    (cpb, src_w [NC,128,*], dst_w [NC,128,*], dstf [NC,nb,128,cpb])."""
    src = np.asarray(edge[0]).astype(np.int64)
    dst = np.asarray(edge[1]).astype(np.int64)
    percore = []
    maxcnt = 0
    for k in range(cfg.ncores):
        m = (dst >= k * cfg.nsr) & (dst < (k + 1) * cfg.nsr)
        s = src[m]
        d = dst[m] - k * cfg.nsr
        o = np.argsort(d, kind="stable")
        s, d = s[o], d[o]
        bc = np.bincount(d // P, minlength=cfg.nb)
        percore.append((s, d, bc))
        maxcnt = max(maxcnt, int(bc.max()))
    cpb = max(1, (maxcnt + P - 1) // P)
    epc = cfg.nb * cpb * P
    pad_row = cfg.ns - 1
    src_rows = np.full((cfg.ncores, epc), pad_row, np.int64)
    dst_rows = np.full((cfg.ncores, epc), pad_row, np.int64)
    dstf = np.full((cfg.ncores, cfg.nb, P, cpb), -1.0, np.float32)
    for k, (s, d, bc) in enumerate(percore):
        starts = np.concatenate([[0], np.cumsum(bc)])
        srow = (s // cfg.nsr) * cfg.ns + (s % cfg.nsr)
        for b in range(cfg.nb):
            cnt = int(bc[b])
            if cnt == 0:
                continue
            base = b * cpb * P
            sl = slice(int(starts[b]), int(starts[b]) + cnt)
            src_rows[k, base:base + cnt] = srow[sl]
            dst_rows[k, base:base + cnt] = d[sl]
            j = np.arange(cnt)
            dstf[k, b, j % P, j // P] = (d[sl] - b * P).astype(np.float32)
    src_w = np.stack([_wrap_idx(src_rows[k]) for k in range(cfg.ncores)])
    dst_w = np.stack([_wrap_idx(dst_rows[k]) for k in range(cfg.ncores)])
    return cpb, src_w, dst_w, dstf


def _f32(x):
    return np.ascontiguousarray(np.asarray(x), dtype=np.float32)


def _bcast(v):
    v = _f32(v)
    return np.ascontiguousarray(np.broadcast_to(v[None, :], (P, v.shape[0])))


def _ktiles(w):
    """[din, m] -> list of [<=128, m] row tiles."""
    w = _f32(w)
    return [np.ascontiguousarray(w[k * P:(k + 1) * P])
            for k in range((w.shape[0] + P - 1) // P)]


def _prep_weights(params, cfg):
    H, C = cfg.heads, cfg.hid
    out = {}
    out["w_pre_u"] = _f32(params["pre_u"][0])
    out["b_pre_u"] = _bcast(params["pre_u"][1])
    out["w_pre_i"] = _f32(params["pre_i"][0])
    out["b_pre_i"] = _bcast(params["pre_i"][1])

    def vproj(W, a):
        W = _f32(W); a = _f32(a)
        return np.stack([W[:, h * C:(h + 1) * C] @ a[h] for h in range(H)], 1)

    for l, lp in enumerate(params["layers"]):
        Wui, asui, adui, bui = lp["ui"]
        Wiu, asiu, adiu, biu = lp["iu"]
        wp_u = np.concatenate([_f32(Wui), vproj(Wui, asui), vproj(Wiu, adiu)], 1)
        wp_i = np.concatenate([_f32(Wiu), vproj(Wiu, asiu), vproj(Wui, adui)], 1)
        for k, t in enumerate(_ktiles(wp_u)):
            out[f"wp_u_{l}_{k}"] = t
        for k, t in enumerate(_ktiles(wp_i)):
            out[f"wp_i_{l}_{k}"] = t
        out[f"bias_ui_{l}"] = _bcast(bui)
        out[f"bias_iu_{l}"] = _bcast(biu)
    for side in ("u", "i"):
        p0, b0, p1, b1 = params[f"post_{side}"]
        for k, t in enumerate(_ktiles(p0)):
            out[f"p0_{side}_{k}"] = t
        out[f"b0_{side}"] = _bcast(b0)
        out[f"p1_{side}"] = _f32(p1)
        out[f"b1_{side}"] = _bcast(b1)
    out["iota"] = np.ascontiguousarray(
        np.broadcast_to(np.arange(P, dtype=np.float32)[None, :], (P, P)))
    out["ident"] = np.eye(P, dtype=np.float32)
    return out


def _node_slices(x, cfg):
    """Full [n, f] -> per-core transposed padded [f, ns]."""
    x = _f32(x)
    outs = []
    for k in range(cfg.ncores):
        sl = x[k * cfg.nsr:(k + 1) * cfg.nsr]
        pad = np.zeros((cfg.ns, x.shape[1]), np.float32)
        pad[:sl.shape[0]] = sl
        outs.append(np.ascontiguousarray(pad.T))
    return outs


def _build(cfg, cpb_ui, cpb_iu, wshapes):
    import concourse.bass as bass
    import concourse.bacc as bacc
    import concourse.tile as tile
    import concourse.mybir as mybir

    f32 = mybir.dt.float32
    i16 = mybir.dt.int16
    AL = mybir.AluOpType
    AF = mybir.ActivationFunctionType
    nb, ns, d1, tblw, H, C = cfg.nb, cfg.ns, cfg.d1, cfg.tblw, cfg.heads, cfg.hid

    nc = bacc.Bacc("TRN2", target_bir_lowering=False, debug=False,
                   num_devices=cfg.ncores)
    I = {}

    def inp(name, shape, dt=f32):
        I[name] = nc.dram_tensor(name, list(shape), dt, kind="ExternalInput").ap()

    inp("xT_u", [cfg.f_in, ns])
    inp("xT_i", [cfg.f_in, ns])
    epc_ui, epc_iu = nb * cpb_ui * P, nb * cpb_iu * P
    inp("sidx_ui", [P, epc_ui // 16], i16)
    inp("didx_ui", [P, epc_ui // 16], i16)
    inp("dstf_ui", [nb, P, cpb_ui])
    inp("sidx_iu", [P, epc_iu // 16], i16)
    inp("didx_iu", [P, epc_iu // 16], i16)
    inp("dstf_iu", [nb, P, cpb_iu])
    for name, arr_shape in wshapes.items():
        inp(name, list(arr_shape))

    out_u = nc.dram_tensor("out_u", [ns, cfg.hid], f32, kind="ExternalOutput").ap()
    out_i = nc.dram_tensor("out_i", [ns, cfg.hid], f32, kind="ExternalOutput").ap()

    rg = [list(range(cfg.ncores))]

    with tile.TileContext(nc) as tc:
        with tc.tile_pool(name="wsb", bufs=1) as wsb, \
             tc.tile_pool(name="sb", bufs=3) as sb, \
             tc.tile_pool(name="gp", bufs=2) as gp, \
             tc.tile_pool(name="ps", bufs=2, space="PSUM") as ps, \
             tc.tile_pool(name="dr", bufs=1, space="DRAM") as dr, \
             tc.tile_pool(name="tdr", bufs=2, space="DRAM") as tdr, \
             tc.tile_pool(name="adr", bufs=2, space="DRAM") as adr:

            # resident weights / constants / indices
            WT = {}
            for name in list(wshapes.keys()) + [
                    "sidx_ui", "didx_ui", "sidx_iu", "didx_iu"]:
                src_ap = I[name]
                dt = i16 if name.startswith(("sidx", "didx")) else f32
                t = wsb.tile(list(src_ap.shape), dt, name=f"w_{name}",
                             tag=f"w_{name}")
                nc.sync.dma_start(t[:], src_ap[:])
                WT[name] = t

            ag_u = dr.tile([ns, tblw], f32, name="ag_u", tag="ag_u")
            ag_i = dr.tile([ns, tblw], f32, name="ag_i", tag="ag_i")
            ed_ui = dr.tile([ns, cfg.edw], f32, name="ed_ui", tag="ed_ui")
            ed_iu = dr.tile([ns, cfg.edw], f32, name="ed_iu", tag="ed_iu")

            def preproj(xname, wname, bname, a_dst):
                for t in range(nb):
                    xt = sb.tile([P, P], f32, name="pp_x", tag="pp_x")
                    nc.sync.dma_start(xt[:], I[xname][:, t * P:(t + 1) * P])
                    pst = ps.tile([P, cfg.hid], f32, name="pp_ps", tag="ps_mm")
                    nc.tensor.matmul(out=pst[:], lhsT=xt[:], rhs=WT[wname][:],
                                     start=True, stop=True)
                    ot = sb.tile([P, cfg.hid], f32, name="pp_o", tag="pp_o")
                    nc.vector.tensor_add(out=ot[:], in0=pst[:], in1=WT[bname][:])
                    nc.scalar.activation(out=ot[:], in_=ot[:], func=AF.Relu)
                    nc.sync.dma_start(a_dst[t * P:(t + 1) * P, :], ot[:])

            def transpose_tiles(src_tile, din, tagbase):
                """[P, din] sbuf -> aT sbuf [P, nk*P] with aT_k in col block k."""
                nk = (din + P - 1) // P
                aT = sb.tile([P, nk * P], f32, name=f"{tagbase}_aT",
                             tag=f"{tagbase}_aT")
                for k in range(nk):
                    kw = min(P, din - k * P)
                    tp = ps.tile([P, P], f32, name=f"{tagbase}_tp", tag="ps_tp")
                    nc.tensor.transpose(out=tp[:kw, :],
                                        in_=src_tile[:, k * P:k * P + kw],
                                        identity=WT["ident"][:])
                    nc.vector.tensor_copy(out=aT[:kw, k * P:(k + 1) * P],
                                          in_=tp[:kw, :])
                return aT, nk

            def node_phase(a_src, din, wpbase, ag_dst, ed_dst):
                nk = (din + P - 1) // P
                for t in range(nb):
                    at = sb.tile([P, din], f32, name="np_a", tag="np_a")
                    nc.sync.dma_start(at[:], a_src[t * P:(t + 1) * P, :])
                    aT, _ = transpose_tiles(at, din, "np")
                    hs = ps.tile([P, d1 + 8], f32, name="np_hs", tag="ps_mm")
                    for k in range(nk):
                        kw = min(P, din - k * P)
                        nc.tensor.matmul(out=hs[:],
                                         lhsT=aT[:kw, k * P:(k + 1) * P],
                                         rhs=WT[f"{wpbase}_{k}"][:],
                                         start=(k == 0), stop=(k == nk - 1))
                    agt = sb.tile([P, tblw], f32, name="np_ag", tag="np_ag")
                    nc.vector.tensor_copy(out=agt[:, 0:d1 + 4], in_=hs[:, 0:d1 + 4])
                    nc.vector.memset(agt[:, d1 + 4:tblw], 0.0)
                    nc.sync.dma_start(ag_dst[t * P:(t + 1) * P, :], agt[:])
                    edt = sb.tile([P, cfg.edw], f32, name="np_ed", tag="np_ed")
                    nc.vector.tensor_copy(out=edt[:, 0:4], in_=hs[:, d1 + 4:d1 + 8])
                    nc.vector.memset(edt[:, 4:cfg.edw], 0.0)
                    nc.sync.dma_start(ed_dst[t * P:(t + 1) * P, :], edt[:])

            def edge_phase(cpb, table, sidx, didx, dstf_name, ed_tbl, bias_name,
                           a_dst, gtag):
                epb = cpb * P
                for b in range(nb):
                    g = gp.tile([P, cpb, tblw], f32, name=f"g_{gtag}",
                                tag=f"g_{gtag}")
                    nc.gpsimd.dma_gather(
                        g[:], table[:],
                        WT[sidx][:, b * (epb // 16):(b + 1) * (epb // 16)],
                        num_idxs=epb, num_idxs_reg=epb, elem_size=tblw,
                        single_packet=False)
                    edg = gp.tile([P, cpb, cfg.edw], f32, name=f"edg_{gtag}",
                                  tag=f"edg_{gtag}")
                    nc.gpsimd.dma_gather(
                        edg[:], ed_tbl[:],
                        WT[didx][:, b * (epb // 16):(b + 1) * (epb // 16)],
                        num_idxs=epb, num_idxs_reg=epb, elem_size=cfg.edw,
                        single_packet=False)
                    dstv = sb.tile([P, cpb], f32, name="eg_dst", tag="eg_dst")
                    nc.sync.dma_start(dstv[:], I[dstf_name][b, :, :])
                    wv = sb.tile([P, cpb * 4], f32, name="eg_w", tag="eg_w")
                    wv3 = wv[:].rearrange("p (c h) -> p c h", h=4)
                    nc.vector.tensor_tensor(out=wv3, in0=g[:, :, d1:d1 + 4],
                                            in1=edg[:, :, 0:4], op=AL.add)
                    tmp = sb.tile([P, cpb * 4], f32, name="eg_t", tag="eg_t")
                    nc.vector.tensor_scalar_mul(out=tmp[:], in0=wv[:],
                                                scalar1=float(cfg.neg))
                    nc.vector.tensor_tensor(out=wv[:], in0=wv[:], in1=tmp[:],
                                            op=AL.max)
                    nc.scalar.activation(out=wv[:], in_=wv[:], func=AF.Exp)
                    acc = ps.tile([P, d1], f32, name="eg_ps", tag="ps_mm")
                    den = ps.tile([P, 4], f32, name="eg_den", tag="ps_den")
                    for c in range(cpb):
                        mask = sb.tile([P, P], f32, name="eg_m", tag="eg_m")
                        nc.vector.tensor_tensor(
                            out=mask[:],
                            in0=dstv[:, c:c + 1].to_broadcast([P, P]),
                            in1=WT["iota"][:], op=AL.is_equal)
                        gv = g[:, c, 0:d1].rearrange("p (h c) -> p h c", h=H)
                        nc.vector.tensor_tensor(
                            out=gv, in0=gv,
                            in1=wv[:, c * 4:(c + 1) * 4].to_broadcast([P, H, C]),
                            op=AL.mult)
                        nc.tensor.matmul(out=acc[:], lhsT=mask[:],
                                         rhs=g[:, c, 0:d1],
                                         start=(c == 0), stop=(c == cpb - 1))
                        nc.tensor.matmul(out=den[:], lhsT=mask[:],
                                         rhs=wv[:, c * 4:(c + 1) * 4],
                                         start=(c == 0), stop=(c == cpb - 1))
                    rec = sb.tile([P, 4], f32, name="eg_r", tag="eg_r")
                    nc.vector.tensor_scalar_add(out=rec[:], in0=den[:],
                                                scalar1=1e-16)
                    nc.vector.reciprocal(out=rec[:], in_=rec[:])
                    ot = sb.tile([P, d1], f32, name="eg_o", tag="eg_o")
                    for h in range(H):
                        nc.vector.tensor_scalar_mul(
                            out=ot[:, h * C:(h + 1) * C],
                            in0=acc[:, h * C:(h + 1) * C],
                            scalar1=rec[:, h:h + 1])
                    nc.vector.tensor_add(out=ot[:], in0=ot[:],
                                         in1=WT[bias_name][:])
                    nc.scalar.activation(out=ot[:], in_=ot[:], func=AF.Relu)
                    nc.sync.dma_start(a_dst[b * P:(b + 1) * P, :], ot[:])

            def post_mlp(a_src, side, out_dst):
                for t in range(nb):
                    at = sb.tile([P, d1], f32, name="pm_a", tag="np_a")
                    nc.sync.dma_start(at[:], a_src[t * P:(t + 1) * P, :])
                    aT, nk = transpose_tiles(at, d1, "pm")
                    m1p = ps.tile([P, cfg.hid], f32, name="pm_ps1", tag="ps_mm")
                    for k in range(nk):
                        nc.tensor.matmul(out=m1p[:],
                                         lhsT=aT[:, k * P:(k + 1) * P],
                                         rhs=WT[f"p0_{side}_{k}"][:],
                                         start=(k == 0), stop=(k == nk - 1))
                    m1 = sb.tile([P, cfg.hid], f32, name="pm_m1", tag="pm_m1")
                    nc.vector.tensor_add(out=m1[:], in0=m1p[:],
                                         in1=WT[f"b0_{side}"][:])
                    nc.scalar.activation(out=m1[:], in_=m1[:], func=AF.Relu)
                    tp = ps.tile([P, P], f32, name="pm_tp", tag="ps_tp")
                    nc.tensor.transpose(out=tp[:cfg.hid, :], in_=m1[:],
                                        identity=WT["ident"][:])
                    m1T = sb.tile([cfg.hid, P], f32, name="pm_m1T", tag="pm_m1T")
                    nc.vector.tensor_copy(out=m1T[:], in_=tp[:cfg.hid, :])
                    op2 = ps.tile([P, cfg.hid], f32, name="pm_ps2", tag="ps_mm")
                    nc.tensor.matmul(out=op2[:], lhsT=m1T[:],
                                     rhs=WT[f"p1_{side}"][:],
                                     start=True, stop=True)
                    ob = sb.tile([P, cfg.hid], f32, name="pm_o", tag="pp_o")
                    nc.vector.tensor_add(out=ob[:], in0=op2[:],
                                         in1=WT[f"b1_{side}"][:])
                    nc.sync.dma_start(out_dst[t * P:(t + 1) * P, :], ob[:])

            a_u = adr.tile([ns, cfg.hid], f32, name="a_u0", tag="a_u")
            a_i = adr.tile([ns, cfg.hid], f32, name="a_i0", tag="a_i")
            preproj("xT_u", "w_pre_u", "b_pre_u", a_u)
            preproj("xT_i", "w_pre_i", "b_pre_i", a_i)
            for l in range(cfg.nl):
                din = cfg.hid if l == 0 else d1
                table_u = tdr.tile([cfg.tbl_rows, tblw], f32,
                                   addr_space="Shared", name=f"table_u{l}",
                                   tag="table_u")
                table_i = tdr.tile([cfg.tbl_rows, tblw], f32,
                                   addr_space="Shared", name=f"table_i{l}",
                                   tag="table_i")
                node_phase(a_u, din, f"wp_u_{l}", ag_u, ed_iu)
                node_phase(a_i, din, f"wp_i_{l}", ag_i, ed_ui)
                nc.gpsimd.collective_compute(
                    "AllGather", mybir.AluOpType.bypass, replica_groups=rg,
                    ins=[ag_u[:].opt()], outs=[table_u[:].opt()])
                nc.gpsimd.collective_compute(
                    "AllGather", mybir.AluOpType.bypass, replica_groups=rg,
                    ins=[ag_i[:].opt()], outs=[table_i[:].opt()])
                a_i2 = adr.tile([ns, d1], f32, name=f"a_i{l + 1}", tag="a_i")
                edge_phase(cpb_ui, table_u, "sidx_ui", "didx_ui", "dstf_ui",
                           ed_ui, f"bias_ui_{l}", a_i2, "ui")
                a_u2 = adr.tile([ns, d1], f32, name=f"a_u{l + 1}", tag="a_u")
                edge_phase(cpb_iu, table_i, "sidx_iu", "didx_iu", "dstf_iu",
                           ed_iu, f"bias_iu_{l}", a_u2, "iu")
                a_u, a_i = a_u2, a_i2
            post_mlp(a_u, "u", out_u)
            post_mlp(a_i, "i", out_i)

    nc.compile()
    return nc


def _prepare(x_user, x_item, edge_ui, edge_iu, params, cfg):
    cpb_ui, sidx_ui, didx_ui, dstf_ui = _prep_dir(np.asarray(edge_ui), cfg)
    cpb_iu, sidx_iu, didx_iu, dstf_iu = _prep_dir(np.asarray(edge_iu), cfg)
    weights = _prep_weights(params, cfg)
    xT_u = _node_slices(x_user, cfg)
    xT_i = _node_slices(x_item, cfg)
    in_maps = []
    for k in range(cfg.ncores):
        m = {
            "xT_u": xT_u[k], "xT_i": xT_i[k],
            "sidx_ui": sidx_ui[k], "didx_ui": didx_ui[k],
            "dstf_ui": np.ascontiguousarray(dstf_ui[k]),
            "sidx_iu": sidx_iu[k], "didx_iu": didx_iu[k],
            "dstf_iu": np.ascontiguousarray(dstf_iu[k]),
        }
        m.update(weights)
        in_maps.append(m)
    wshapes = {k: v.shape for k, v in weights.items()}
    return cpb_ui, cpb_iu, wshapes, in_maps


def _install_ntff_hook():
    """Provide antenv.axon_hooks via ctypes when the image lacks it."""
    import types
    try:
        from antenv.axon_hooks import get_axon_ntff_profile_hook  # noqa: F401
        return
    except ImportError:
        pass
    try:
        from trn_agent_boot.trn_boot import _ntff_profile_via_ctypes
        hook = _ntff_profile_via_ctypes('/opt/axon/libaxon_pjrt.so')
    except Exception:
        return
    mod = types.ModuleType('antenv.axon_hooks')
    mod.get_axon_ntff_profile_hook = lambda: hook
    sys.modules['antenv.axon_hooks'] = mod


def _run(x_user, x_item, edge_ui, edge_iu, params, cfg=None, trace=False):
    from concourse import bass_utils
    if trace:
        _install_ntff_hook()
    cfg = cfg or Cfg()
    cpb_ui, cpb_iu, wshapes, in_maps = _prepare(
        x_user, x_item, edge_ui, edge_iu, params, cfg)
    nc = _build(cfg, cpb_ui, cpb_iu, wshapes)
    res = bass_utils.run_bass_kernel_spmd(
        nc, in_maps, core_ids=list(range(cfg.ncores)), trace=trace)
    out_u = np.concatenate(
        [res.results[k]["out_u"][:cfg.nsr] for k in range(cfg.ncores)], 0)
    out_i = np.concatenate(
        [res.results[k]["out_i"][:cfg.nsr] for k in range(cfg.ncores)], 0)
    return (out_u, out_i), res


def kernel(x_user, x_item, edge_ui, edge_iu, params):
    (out_u, out_i), _ = _run(x_user, x_item, edge_ui, edge_iu, params)
    return out_u, out_i


# revision 11
# speedup vs baseline: 1.4125x; 1.4125x over previous
"""Bipartite 3-layer GAT (user<->item) on 8 Trainium2 NeuronCores.

Strategy:
- Destination-range sharding: core k owns user-nodes and item-nodes
  [k*2500, (k+1)*2500) as edge destinations. All edges whose dst falls in
  that range are processed by core k (sorted by dst on host).
- Node phase is sharded by node rows; each core computes its slice of
  h = a @ W (plus per-head attention projections es/ed), then an
  AllGather replicates the (h|es) table so every core can gather rows
  for arbitrary source nodes.
- Edge phase: per 128-dst block, batched dma_gather of source rows
  (h|es) and of per-dst ed rows, softmax weights w = exp(leaky(es+ed))
  (the segment-max cancels mathematically and scores are small, so it
  is skipped), messages scaled by w, and a one-hot matmul segment-sum
  accumulated in PSUM. Softmax denominators accumulate in the same
  PSUM tile via a second matmul against w.
- Host does index preprocessing only (shard/sort/pad); all FLOPs and
  data movement run on device.
"""
import sys

for _p in ("/opt/trn_rl_repo",):
    if _p not in sys.path:
        sys.path.insert(0, _p)

import numpy as np

P = 128


class Cfg:
    def __init__(self, n=20000, e=320000, f_in=128, hid=64, heads=4, nl=3,
                 neg=0.2, ncores=8):
        self.n, self.e, self.f_in, self.hid = n, e, f_in, hid
        self.heads, self.nl, self.neg, self.ncores = heads, nl, neg, ncores
        self.nsr = n // ncores                    # raw nodes per core
        self.nb = (self.nsr + P - 1) // P         # dst blocks per core
        self.ns = self.nb * P                     # padded nodes per core
        self.d1 = heads * hid                     # 256
        self.tblw = self.d1 + 64                  # [hs d1 | es 4 | pad] 256B-mult
        self.tbl_rows = ncores * self.ns
        self.edw = 64                             # ed table width (256B rows)


def _wrap_idx(ix):
    """[T] int -> [128, T//16] int16 (16-partition wrap, replicated 8x)."""
    assert ix.shape[0] % 16 == 0
    w = ix.reshape(-1, 16).T.astype(np.int16)
    return np.ascontiguousarray(np.tile(w, (8, 1)))


def _prep_dir(edge, cfg):
    """Shard+sort one edge direction by dst range. Returns
    (cpb, src_w [NC,128,*], dst_w [NC,128,*], dstf [NC,nb,128,cpb])."""
    src = np.asarray(edge[0]).astype(np.int64)
    dst = np.asarray(edge[1]).astype(np.int64)
    percore = []
    maxcnt = 0
    for k in range(cfg.ncores):
        m = (dst >= k * cfg.nsr) & (dst < (k + 1) * cfg.nsr)
        s = src[m]
        d = dst[m] - k * cfg.nsr
        o = np.argsort(d, kind="stable")
        s, d = s[o], d[o]
        bc = np.bincount(d // P, minlength=cfg.nb)
        percore.append((s, d, bc))
        maxcnt = max(maxcnt, int(bc.max()))
    cpb = max(1, (maxcnt + P - 1) // P)
    epc = cfg.nb * cpb * P
    pad_row = cfg.ns - 1
    src_rows = np.full((cfg.ncores, epc), pad_row, np.int64)
    dst_rows = np.full((cfg.ncores, epc), pad_row, np.int64)
    dstf = np.full((cfg.ncores, cfg.nb, P, cpb), -1.0, np.float32)
    for k, (s, d, bc) in enumerate(percore):
        starts = np.concatenate([[0], np.cumsum(bc)])
        srow = (s // cfg.nsr) * cfg.ns + (s % cfg.nsr)
        for b in range(cfg.nb):
            cnt = int(bc[b])
            if cnt == 0:
                continue
            base = b * cpb * P
            sl = slice(int(starts[b]), int(starts[b]) + cnt)
            src_rows[k, base:base + cnt] = srow[sl]
            dst_rows[k, base:base + cnt] = d[sl]
            j = np.arange(cnt)
            dstf[k, b, j % P, j // P] = (d[sl] - b * P).astype(np.float32)
    src_w = np.stack([_wrap_idx(src_rows[k]) for k in range(cfg.ncores)])
    dst_w = np.stack([_wrap_idx(dst_rows[k]) for k in range(cfg.ncores)])
    return cpb, src_w, dst_w, dstf


def _f32(x):
    return np.ascontiguousarray(np.asarray(x), dtype=np.float32)


def _bcast(v):
    v = _f32(v)
    return np.ascontiguousarray(np.broadcast_to(v[None, :], (P, v.shape[0])))


def _ktiles(w):
    """[din, m] -> list of [<=128, m] row tiles."""
    w = _f32(w)
    return [np.ascontiguousarray(w[k * P:(k + 1) * P])
            for k in range((w.shape[0] + P - 1) // P)]


def _prep_weights(params, cfg):
    H, C = cfg.heads, cfg.hid
    out = {}
    out["w_pre_u"] = _f32(params["pre_u"][0])
    out["b_pre_u"] = _bcast(params["pre_u"][1])
    out["w_pre_i"] = _f32(params["pre_i"][0])
    out["b_pre_i"] = _bcast(params["pre_i"][1])

    def vproj(W, a):
        W = _f32(W); a = _f32(a)
        return np.stack([W[:, h * C:(h + 1) * C] @ a[h] for h in range(H)], 1)

    for l, lp in enumerate(params["layers"]):
        Wui, asui, adui, bui = lp["ui"]
        Wiu, asiu, adiu, biu = lp["iu"]
        wp_u = np.concatenate([_f32(Wui), vproj(Wui, asui), vproj(Wiu, adiu)], 1)
        wp_i = np.concatenate([_f32(Wiu), vproj(Wiu, asiu), vproj(Wui, adui)], 1)
        for k, t in enumerate(_ktiles(wp_u)):
            out[f"wp_u_{l}_{k}"] = t
        for k, t in enumerate(_ktiles(wp_i)):
            out[f"wp_i_{l}_{k}"] = t
        out[f"bias_ui_{l}"] = _bcast(bui)
        out[f"bias_iu_{l}"] = _bcast(biu)
    for side in ("u", "i"):
        p0, b0, p1, b1 = params[f"post_{side}"]
        for k, t in enumerate(_ktiles(p0)):
            out[f"p0_{side}_{k}"] = t
        out[f"b0_{side}"] = _bcast(b0)
        out[f"p1_{side}"] = _f32(p1)
        out[f"b1_{side}"] = _bcast(b1)
    out["iota"] = np.ascontiguousarray(
        np.broadcast_to(np.arange(P, dtype=np.float32)[None, :], (P, P)))
    out["ident"] = np.eye(P, dtype=np.float32)
    return out


def _node_slices(x, cfg):
    """Full [n, f] -> per-core transposed padded [f, ns]."""
    x = _f32(x)
    outs = []
    for k in range(cfg.ncores):
        sl = x[k * cfg.nsr:(k + 1) * cfg.nsr]
        pad = np.zeros((cfg.ns, x.shape[1]), np.float32)
        pad[:sl.shape[0]] = sl
        outs.append(np.ascontiguousarray(pad.T))
    return outs


def _build(cfg, cpb_ui, cpb_iu, wshapes):
    import concourse.bass as bass
    import concourse.bacc as bacc
    import concourse.tile as tile
    import concourse.mybir as mybir

    f32 = mybir.dt.float32
    bf16 = mybir.dt.bfloat16
    i16 = mybir.dt.int16
    AL = mybir.AluOpType
    AF = mybir.ActivationFunctionType
    nb, ns, d1, tblw, H, C = cfg.nb, cfg.ns, cfg.d1, cfg.tblw, cfg.heads, cfg.hid

    nc = bacc.Bacc("TRN2", target_bir_lowering=False, debug=False,
                   num_devices=cfg.ncores)
    I = {}

    def inp(name, shape, dt=f32):
        I[name] = nc.dram_tensor(name, list(shape), dt, kind="ExternalInput").ap()

    inp("xT_u", [cfg.f_in, ns])
    inp("xT_i", [cfg.f_in, ns])
    epc_ui, epc_iu = nb * cpb_ui * P, nb * cpb_iu * P
    inp("sidx_ui", [P, epc_ui // 16], i16)
    inp("didx_ui", [P, epc_ui // 16], i16)
    inp("dstf_ui", [nb, P, cpb_ui])
    inp("sidx_iu", [P, epc_iu // 16], i16)
    inp("didx_iu", [P, epc_iu // 16], i16)
    inp("dstf_iu", [nb, P, cpb_iu])
    for name, arr_shape in wshapes.items():
        inp(name, list(arr_shape))

    out_u = nc.dram_tensor("out_u", [ns, cfg.hid], f32, kind="ExternalOutput").ap()
    out_i = nc.dram_tensor("out_i", [ns, cfg.hid], f32, kind="ExternalOutput").ap()

    rg = [list(range(cfg.ncores))]

    with tile.TileContext(nc) as tc:
        with tc.tile_pool(name="wsb", bufs=1) as wsb, \
             tc.tile_pool(name="sb", bufs=3) as sb, \
             tc.tile_pool(name="gp", bufs=2) as gp, \
             tc.tile_pool(name="ps", bufs=2, space="PSUM") as ps, \
             tc.tile_pool(name="dr", bufs=1, space="DRAM") as dr, \
             tc.tile_pool(name="tdr", bufs=2, space="DRAM") as tdr, \
             tc.tile_pool(name="adr", bufs=2, space="DRAM") as adr:

            # resident weights / constants / indices
            WT = {}
            for name in list(wshapes.keys()) + [
                    "sidx_ui", "didx_ui", "sidx_iu", "didx_iu"]:
                src_ap = I[name]
                dt = i16 if name.startswith(("sidx", "didx")) else f32
                t = wsb.tile(list(src_ap.shape), dt, name=f"w_{name}",
                             tag=f"w_{name}")
                nc.sync.dma_start(t[:], src_ap[:])
                WT[name] = t
            ident16 = wsb.tile([P, P], bf16, name="w_ident16", tag="w_ident16")
            nc.vector.tensor_copy(out=ident16[:], in_=WT["ident"][:])
            WT["ident16"] = ident16

            ag_u = dr.tile([ns, tblw], f32, name="ag_u", tag="ag_u")
            ag_i = dr.tile([ns, tblw], f32, name="ag_i", tag="ag_i")
            ed_ui = dr.tile([ns, 4], f32, name="ed_ui", tag="ed_ui")
            ed_iu = dr.tile([ns, 4], f32, name="ed_iu", tag="ed_iu")

            def preproj(xname, wname, bname, a_dst):
                for t in range(nb):
                    xt = sb.tile([P, P], f32, name="pp_x", tag="pp_x")
                    nc.sync.dma_start(xt[:], I[xname][:, t * P:(t + 1) * P])
                    pst = ps.tile([P, cfg.hid], f32, name="pp_ps", tag="ps_mm")
                    nc.tensor.matmul(out=pst[:], lhsT=xt[:], rhs=WT[wname][:],
                                     start=True, stop=True)
                    ot = sb.tile([P, cfg.hid], f32, name="pp_o", tag="pp_o")
                    nc.vector.tensor_add(out=ot[:], in0=pst[:], in1=WT[bname][:])
                    nc.scalar.activation(out=ot[:], in_=ot[:], func=AF.Relu)
                    nc.sync.dma_start(a_dst[t * P:(t + 1) * P, :], ot[:])

            def transpose_tiles(src_tile, din, tagbase):
                """[P, din] sbuf -> aT sbuf [P, nk*P] with aT_k in col block k."""
                nk = (din + P - 1) // P
                aT = sb.tile([P, nk * P], f32, name=f"{tagbase}_aT",
                             tag=f"{tagbase}_aT")
                for k in range(nk):
                    kw = min(P, din - k * P)
                    tp = ps.tile([P, P], f32, name=f"{tagbase}_tp", tag="ps_tp")
                    nc.tensor.transpose(out=tp[:kw, :],
                                        in_=src_tile[:, k * P:k * P + kw],
                                        identity=WT["ident"][:])
                    nc.vector.tensor_copy(out=aT[:kw, k * P:(k + 1) * P],
                                          in_=tp[:kw, :])
                return aT, nk

            def node_phase(a_src, din, wpbase, ag_dst, ed_dst):
                nk = (din + P - 1) // P
                for t in range(nb):
                    at = sb.tile([P, din], f32, name="np_a", tag="np_a")
                    nc.sync.dma_start(at[:], a_src[t * P:(t + 1) * P, :])
                    aT, _ = transpose_tiles(at, din, "np")
                    hs = ps.tile([P, d1 + 8], f32, name="np_hs", tag="ps_mm")
                    for k in range(nk):
                        kw = min(P, din - k * P)
                        nc.tensor.matmul(out=hs[:],
                                         lhsT=aT[:kw, k * P:(k + 1) * P],
                                         rhs=WT[f"{wpbase}_{k}"][:],
                                         start=(k == 0), stop=(k == nk - 1))
                    agt = sb.tile([P, tblw], f32, name="np_ag", tag="np_ag")
                    nc.vector.tensor_copy(out=agt[:, 0:d1 + 4], in_=hs[:, 0:d1 + 4])
                    nc.vector.memset(agt[:, d1 + 4:tblw], 0.0)
                    nc.sync.dma_start(ag_dst[t * P:(t + 1) * P, :], agt[:])
                    edt = sb.tile([P, 4], f32, name="np_ed", tag="np_ed")
                    nc.vector.tensor_copy(out=edt[:], in_=hs[:, d1 + 4:d1 + 8])
                    nc.sync.dma_start(ed_dst[t * P:(t + 1) * P, :], edt[:])

            def edge_phase(cpb, table, sidx, didx, dstf_name, ed_tbl, bias_name,
                           a_dst, gtag):
                epb = cpb * P
                for b in range(nb):
                    g = gp.tile([P, cpb, tblw], f32, name=f"g_{gtag}",
                                tag=f"g_{gtag}")
                    nc.gpsimd.dma_gather(
                        g[:], table[:],
                        WT[sidx][:, b * (epb // 16):(b + 1) * (epb // 16)],
                        num_idxs=epb, num_idxs_reg=epb, elem_size=tblw,
                        single_packet=False)
                    dstv = sb.tile([P, cpb], f32, name="eg_dst", tag="eg_dst")
                    nc.sync.dma_start(dstv[:], I[dstf_name][b, :, :])
                    edb = sb.tile([P, 4], f32, name="eg_edb", tag="eg_edb")
                    nc.sync.dma_start(edb[:], ed_tbl[b * P:(b + 1) * P, :])
                    edb16 = sb.tile([P, 4], bf16, name="eg_edb16", tag="eg_edb16")
                    nc.vector.tensor_copy(out=edb16[:], in_=edb[:])
                    # all chunk masks in one bf16 tile, one batched compare
                    mask = sb.tile([P, cpb, P], bf16, name="eg_m", tag="eg_m")
                    nc.vector.tensor_tensor(
                        out=mask[:],
                        in0=dstv[:].to_broadcast([P, cpb, P]),
                        in1=WT["iota"][:].rearrange("p (o e) -> p o e", o=1)
                            .to_broadcast([P, cpb, P]),
                        op=AL.is_equal)
                    # scores: es (from gather) + ed (expanded via maskT matmul)
                    wv = sb.tile([P, cpb * 4], f32, name="eg_w", tag="eg_w")
                    nc.vector.tensor_copy(
                        out=wv[:].rearrange("p (c h) -> p c h", h=4),
                        in_=g[:, :, d1:d1 + 4])
                    edpe = ps.tile([P, cpb * 4], f32, name="eg_edpe",
                                   tag="ps_den")
                    for c in range(cpb):
                        tp = ps.tile([P, P], bf16, name="eg_tp", tag="ps_tp")
                        nc.tensor.transpose(out=tp[:], in_=mask[:, c, :],
                                            identity=WT["ident16"][:])
                        mT = sb.tile([P, P], bf16, name="eg_mT", tag="eg_mT")
                        nc.scalar.activation(out=mT[:], in_=tp[:], func=AF.Copy)
                        nc.tensor.matmul(out=edpe[:, c * 4:(c + 1) * 4],
                                         lhsT=mT[:], rhs=edb16[:],
                                         start=True, stop=True)
                    nc.vector.tensor_add(out=wv[:], in0=wv[:], in1=edpe[:])
                    tmp = sb.tile([P, cpb * 4], f32, name="eg_t", tag="eg_t")
                    nc.vector.tensor_scalar_mul(out=tmp[:], in0=wv[:],
                                                scalar1=float(cfg.neg))
                    nc.vector.tensor_tensor(out=wv[:], in0=wv[:], in1=tmp[:],
                                            op=AL.max)
                    nc.scalar.activation(out=wv[:], in_=wv[:], func=AF.Exp)
                    # msg16: [msg 256 | w 4] per chunk, bf16
                    msg = gp.tile([P, cpb, d1 + 4], bf16, name=f"msg_{gtag}",
                                  tag=f"msg_{gtag}")
                    nc.vector.tensor_tensor(
                        out=msg[:, :, 0:d1].rearrange("p c (h f) -> p c h f", h=H),
                        in0=g[:, :, 0:d1].rearrange("p c (h f) -> p c h f", h=H),
                        in1=wv[:].rearrange("p (c h) -> p c h", h=4)
                            .to_broadcast([P, cpb, H, C]),
                        op=AL.mult)
                    nc.vector.tensor_copy(
                        out=msg[:, :, d1:d1 + 4],
                        in_=wv[:].rearrange("p (c h) -> p c h", h=4))
                    acc = ps.tile([P, d1 + 4], f32, name="eg_ps", tag="ps_mm")
                    for c in range(cpb):
                        nc.tensor.matmul(out=acc[:], lhsT=mask[:, c, :],
                                         rhs=msg[:, c, :],
                                         start=(c == 0), stop=(c == cpb - 1))
                    rec = sb.tile([P, 4], f32, name="eg_r", tag="eg_r")
                    nc.vector.tensor_scalar_add(out=rec[:], in0=acc[:, d1:d1 + 4],
                                                scalar1=1e-16)
                    nc.vector.reciprocal(out=rec[:], in_=rec[:])
                    ot = sb.tile([P, d1], f32, name="eg_o", tag="eg_o")
                    for h in range(H):
                        nc.vector.tensor_scalar_mul(
                            out=ot[:, h * C:(h + 1) * C],
                            in0=acc[:, h * C:(h + 1) * C],
                            scalar1=rec[:, h:h + 1])
                    nc.vector.tensor_add(out=ot[:], in0=ot[:],
                                         in1=WT[bias_name][:])
                    nc.scalar.activation(out=ot[:], in_=ot[:], func=AF.Relu)
                    nc.sync.dma_start(a_dst[b * P:(b + 1) * P, :], ot[:])

            def post_mlp(a_src, side, out_dst):
                for t in range(nb):
                    at = sb.tile([P, d1], f32, name="pm_a", tag="np_a")
                    nc.sync.dma_start(at[:], a_src[t * P:(t + 1) * P, :])
                    aT, nk = transpose_tiles(at, d1, "pm")
                    m1p = ps.tile([P, cfg.hid], f32, name="pm_ps1", tag="ps_mm")
                    for k in range(nk):
                        nc.tensor.matmul(out=m1p[:],
                                         lhsT=aT[:, k * P:(k + 1) * P],
                                         rhs=WT[f"p0_{side}_{k}"][:],
                                         start=(k == 0), stop=(k == nk - 1))
                    m1 = sb.tile([P, cfg.hid], f32, name="pm_m1", tag="pm_m1")
                    nc.vector.tensor_add(out=m1[:], in0=m1p[:],
                                         in1=WT[f"b0_{side}"][:])
                    nc.scalar.activation(out=m1[:], in_=m1[:], func=AF.Relu)
                    tp = ps.tile([P, P], f32, name="pm_tp", tag="ps_tp")
                    nc.tensor.transpose(out=tp[:cfg.hid, :], in_=m1[:],
                                        identity=WT["ident"][:])
                    m1T = sb.tile([cfg.hid, P], f32, name="pm_m1T", tag="pm_m1T")
                    nc.vector.tensor_copy(out=m1T[:], in_=tp[:cfg.hid, :])
                    op2 = ps.tile([P, cfg.hid], f32, name="pm_ps2", tag="ps_mm")
                    nc.tensor.matmul(out=op2[:], lhsT=m1T[:],
                                     rhs=WT[f"p1_{side}"][:],
                                     start=True, stop=True)
                    ob = sb.tile([P, cfg.hid], f32, name="pm_o", tag="pp_o")
                    nc.vector.tensor_add(out=ob[:], in0=op2[:],
                                         in1=WT[f"b1_{side}"][:])
                    nc.sync.dma_start(out_dst[t * P:(t + 1) * P, :], ob[:])

            a_u = adr.tile([ns, cfg.hid], f32, name="a_u0", tag="a_u")
            a_i = adr.tile([ns, cfg.hid], f32, name="a_i0", tag="a_i")
            preproj("xT_u", "w_pre_u", "b_pre_u", a_u)
            preproj("xT_i", "w_pre_i", "b_pre_i", a_i)
            for l in range(cfg.nl):
                din = cfg.hid if l == 0 else d1
                table_u = tdr.tile([cfg.tbl_rows, tblw], f32,
                                   addr_space="Shared", name=f"table_u{l}",
                                   tag="table_u")
                table_i = tdr.tile([cfg.tbl_rows, tblw], f32,
                                   addr_space="Shared", name=f"table_i{l}",
                                   tag="table_i")
                node_phase(a_u, din, f"wp_u_{l}", ag_u, ed_iu)
                node_phase(a_i, din, f"wp_i_{l}", ag_i, ed_ui)
                nc.gpsimd.collective_compute(
                    "AllGather", mybir.AluOpType.bypass, replica_groups=rg,
                    ins=[ag_u[:].opt()], outs=[table_u[:].opt()])
                nc.gpsimd.collective_compute(
                    "AllGather", mybir.AluOpType.bypass, replica_groups=rg,
                    ins=[ag_i[:].opt()], outs=[table_i[:].opt()])
                a_i2 = adr.tile([ns, d1], f32, name=f"a_i{l + 1}", tag="a_i")
                edge_phase(cpb_ui, table_u, "sidx_ui", "didx_ui", "dstf_ui",
                           ed_ui, f"bias_ui_{l}", a_i2, "ui")
                a_u2 = adr.tile([ns, d1], f32, name=f"a_u{l + 1}", tag="a_u")
                edge_phase(cpb_iu, table_i, "sidx_iu", "didx_iu", "dstf_iu",
                           ed_iu, f"bias_iu_{l}", a_u2, "iu")
                a_u, a_i = a_u2, a_i2
            post_mlp(a_u, "u", out_u)
            post_mlp(a_i, "i", out_i)

    nc.compile()
    return nc


def _prepare(x_user, x_item, edge_ui, edge_iu, params, cfg):
    cpb_ui, sidx_ui, didx_ui, dstf_ui = _prep_dir(np.asarray(edge_ui), cfg)
    cpb_iu, sidx_iu, didx_iu, dstf_iu = _prep_dir(np.asarray(edge_iu), cfg)
    weights = _prep_weights(params, cfg)
    xT_u = _node_slices(x_user, cfg)
    xT_i = _node_slices(x_item, cfg)
    in_maps = []
    for k in range(cfg.ncores):
        m = {
            "xT_u": xT_u[k], "xT_i": xT_i[k],
            "sidx_ui": sidx_ui[k], "didx_ui": didx_ui[k],
            "dstf_ui": np.ascontiguousarray(dstf_ui[k]),
            "sidx_iu": sidx_iu[k], "didx_iu": didx_iu[k],
            "dstf_iu": np.ascontiguousarray(dstf_iu[k]),
        }
        m.update(weights)
        in_maps.append(m)
    wshapes = {k: v.shape for k, v in weights.items()}
    return cpb_ui, cpb_iu, wshapes, in_maps


def _install_ntff_hook():
    """Provide antenv.axon_hooks via ctypes when the image lacks it."""
    import types
    try:
        from antenv.axon_hooks import get_axon_ntff_profile_hook  # noqa: F401
        return
    except ImportError:
        pass
    try:
        from trn_agent_boot.trn_boot import _ntff_profile_via_ctypes
        hook = _ntff_profile_via_ctypes('/opt/axon/libaxon_pjrt.so')
    except Exception:
        return
    mod = types.ModuleType('antenv.axon_hooks')
    mod.get_axon_ntff_profile_hook = lambda: hook
    sys.modules['antenv.axon_hooks'] = mod


def _run(x_user, x_item, edge_ui, edge_iu, params, cfg=None, trace=False):
    from concourse import bass_utils
    if trace:
        _install_ntff_hook()
    cfg = cfg or Cfg()
    cpb_ui, cpb_iu, wshapes, in_maps = _prepare(
        x_user, x_item, edge_ui, edge_iu, params, cfg)
    nc = _build(cfg, cpb_ui, cpb_iu, wshapes)
    res = bass_utils.run_bass_kernel_spmd(
        nc, in_maps, core_ids=list(range(cfg.ncores)), trace=trace)
    out_u = np.concatenate(
        [res.results[k]["out_u"][:cfg.nsr] for k in range(cfg.ncores)], 0)
    out_i = np.concatenate(
        [res.results[k]["out_i"][:cfg.nsr] for k in range(cfg.ncores)], 0)
    return (out_u, out_i), res


def kernel(x_user, x_item, edge_ui, edge_iu, params):
    (out_u, out_i), _ = _run(x_user, x_item, edge_ui, edge_iu, params)
    return out_u, out_i


# revision 14
# speedup vs baseline: 1.5356x; 1.0871x over previous
"""Bipartite 3-layer GAT (user<->item) on 8 Trainium2 NeuronCores.

Strategy:
- Destination-range sharding: core k owns user-nodes and item-nodes
  [k*2500, (k+1)*2500) as edge destinations. All edges whose dst falls in
  that range are processed by core k (sorted by dst on host).
- Node phase is sharded by node rows; each core computes its slice of
  h = a @ W (plus per-head attention projections es/ed), then an
  AllGather replicates the (h|es) table so every core can gather rows
  for arbitrary source nodes.
- Edge phase: per 128-dst block, batched dma_gather of source rows
  (h|es) and of per-dst ed rows, softmax weights w = exp(leaky(es+ed))
  (the segment-max cancels mathematically and scores are small, so it
  is skipped), messages scaled by w, and a one-hot matmul segment-sum
  accumulated in PSUM. Softmax denominators accumulate in the same
  PSUM tile via a second matmul against w.
- Host does index preprocessing only (shard/sort/pad); all FLOPs and
  data movement run on device.
"""
import sys

for _p in ("/opt/trn_rl_repo",):
    if _p not in sys.path:
        sys.path.insert(0, _p)

import numpy as np

P = 128


class Cfg:
    def __init__(self, n=20000, e=320000, f_in=128, hid=64, heads=4, nl=3,
                 neg=0.2, ncores=8):
        self.n, self.e, self.f_in, self.hid = n, e, f_in, hid
        self.heads, self.nl, self.neg, self.ncores = heads, nl, neg, ncores
        self.nsr = n // ncores                    # raw nodes per core
        self.nb = (self.nsr + P - 1) // P         # dst blocks per core
        self.ns = self.nb * P                     # padded nodes per core
        self.d1 = heads * hid                     # 256
        self.tblw = self.d1 + 64                  # [hs d1 | es 4 | pad] 256B-mult
        self.tbl_rows = ncores * self.ns
        self.edw = 64                             # ed table width (256B rows)


def _wrap_idx(ix):
    """[T] int -> [128, T//16] int16 (16-partition wrap, replicated 8x)."""
    assert ix.shape[0] % 16 == 0
    w = ix.reshape(-1, 16).T.astype(np.int16)
    return np.ascontiguousarray(np.tile(w, (8, 1)))


def _prep_dir(edge, cfg):
    """Shard+sort one edge direction by dst range. Returns (cpb, counts[nb],
    src_w [NC,128,*], dstf [NC,nb,128,cpb])."""
    src = np.asarray(edge[0]).astype(np.int64)
    dst = np.asarray(edge[1]).astype(np.int64)
    percore = []
    blockcnt = np.zeros((cfg.ncores, cfg.nb), np.int64)
    for k in range(cfg.ncores):
        m = (dst >= k * cfg.nsr) & (dst < (k + 1) * cfg.nsr)
        s = src[m]
        d = dst[m] - k * cfg.nsr
        o = np.argsort(d, kind="stable")
        s, d = s[o], d[o]
        blockcnt[k] = np.bincount(d // P, minlength=cfg.nb)
        percore.append((s, d))
    cpb = max(1, (int(blockcnt.max()) + P - 1) // P)
    # uniform per-block gather count (same for all cores; baked constant)
    counts = blockcnt.max(axis=0)
    counts = np.minimum(((counts + 15) // 16) * 16, cpb * P)
    counts[0:2] = cpb * P  # first two blocks fully sanitize the SBUF slots
    epc = cfg.nb * cpb * P
    pad_row = cfg.ns - 1
    src_rows = np.full((cfg.ncores, epc), -1, np.int64)
    dstf = np.full((cfg.ncores, cfg.nb, P, cpb), -1.0, np.float32)
    for k, (s, d) in enumerate(percore):
        starts = np.concatenate([[0], np.cumsum(blockcnt[k])])
        srow = (s // cfg.nsr) * cfg.ns + (s % cfg.nsr)
        for b in range(cfg.nb):
            cnt = int(blockcnt[k][b])
            base = b * cpb * P
            sl = slice(int(starts[b]), int(starts[b]) + cnt)
            src_rows[k, base:base + cnt] = srow[sl]
            # valid pads up to the uniform count, -1 beyond
            src_rows[k, base + cnt:base + int(counts[b])] = pad_row
            j = np.arange(cnt)
            dstf[k, b, j % P, j // P] = (d[sl] - b * P).astype(np.float32)
    src_w = np.stack([_wrap_idx(src_rows[k]) for k in range(cfg.ncores)])
    return cpb, counts, src_w, dstf


def _f32(x):
    return np.ascontiguousarray(np.asarray(x), dtype=np.float32)


def _bcast(v):
    v = _f32(v)
    return np.ascontiguousarray(np.broadcast_to(v[None, :], (P, v.shape[0])))


def _ktiles(w):
    """[din, m] -> list of [<=128, m] row tiles."""
    w = _f32(w)
    return [np.ascontiguousarray(w[k * P:(k + 1) * P])
            for k in range((w.shape[0] + P - 1) // P)]


def _prep_weights(params, cfg):
    H, C = cfg.heads, cfg.hid
    out = {}
    out["w_pre_u"] = _f32(params["pre_u"][0])
    out["b_pre_u"] = _bcast(params["pre_u"][1])
    out["w_pre_i"] = _f32(params["pre_i"][0])
    out["b_pre_i"] = _bcast(params["pre_i"][1])

    def vproj(W, a):
        W = _f32(W); a = _f32(a)
        return np.stack([W[:, h * C:(h + 1) * C] @ a[h] for h in range(H)], 1)

    for l, lp in enumerate(params["layers"]):
        Wui, asui, adui, bui = lp["ui"]
        Wiu, asiu, adiu, biu = lp["iu"]
        wp_u = np.concatenate([_f32(Wui), vproj(Wui, asui), vproj(Wiu, adiu)], 1)
        wp_i = np.concatenate([_f32(Wiu), vproj(Wiu, asiu), vproj(Wui, adui)], 1)
        for k, t in enumerate(_ktiles(wp_u)):
            out[f"wp_u_{l}_{k}"] = t
        for k, t in enumerate(_ktiles(wp_i)):
            out[f"wp_i_{l}_{k}"] = t
        out[f"bias_ui_{l}"] = _bcast(bui)
        out[f"bias_iu_{l}"] = _bcast(biu)
    for side in ("u", "i"):
        p0, b0, p1, b1 = params[f"post_{side}"]
        for k, t in enumerate(_ktiles(p0)):
            out[f"p0_{side}_{k}"] = t
        out[f"b0_{side}"] = _bcast(b0)
        out[f"p1_{side}"] = _f32(p1)
        out[f"b1_{side}"] = _bcast(b1)
    out["iota"] = np.ascontiguousarray(
        np.broadcast_to(np.arange(P, dtype=np.float32)[None, :], (P, P)))
    out["ident"] = np.eye(P, dtype=np.float32)
    return out


def _node_slices(x, cfg):
    """Full [n, f] -> per-core transposed padded [f, ns]."""
    x = _f32(x)
    outs = []
    for k in range(cfg.ncores):
        sl = x[k * cfg.nsr:(k + 1) * cfg.nsr]
        pad = np.zeros((cfg.ns, x.shape[1]), np.float32)
        pad[:sl.shape[0]] = sl
        outs.append(np.ascontiguousarray(pad.T))
    return outs


def _build(cfg, cpb_ui, cpb_iu, cnt_ui, cnt_iu, wshapes):
    import concourse.bass as bass
    import concourse.bacc as bacc
    import concourse.tile as tile
    import concourse.mybir as mybir

    f32 = mybir.dt.float32
    bf16 = mybir.dt.bfloat16
    i16 = mybir.dt.int16
    AL = mybir.AluOpType
    AF = mybir.ActivationFunctionType
    nb, ns, d1, tblw, H, C = cfg.nb, cfg.ns, cfg.d1, cfg.tblw, cfg.heads, cfg.hid

    nc = bacc.Bacc("TRN2", target_bir_lowering=False, debug=False,
                   num_devices=cfg.ncores)
    I = {}

    def inp(name, shape, dt=f32):
        I[name] = nc.dram_tensor(name, list(shape), dt, kind="ExternalInput").ap()

    inp("xT_u", [cfg.f_in, ns])
    inp("xT_i", [cfg.f_in, ns])
    epc_ui, epc_iu = nb * cpb_ui * P, nb * cpb_iu * P
    inp("sidx_ui", [P, epc_ui // 16], i16)
    inp("dstf_ui", [nb, P, cpb_ui])
    inp("sidx_iu", [P, epc_iu // 16], i16)
    inp("dstf_iu", [nb, P, cpb_iu])
    for name, arr_shape in wshapes.items():
        inp(name, list(arr_shape))

    out_u = nc.dram_tensor("out_u", [ns, cfg.hid], f32, kind="ExternalOutput").ap()
    out_i = nc.dram_tensor("out_i", [ns, cfg.hid], f32, kind="ExternalOutput").ap()

    rg = [list(range(cfg.ncores))]

    with tile.TileContext(nc) as tc:
        with tc.tile_pool(name="wsb", bufs=1) as wsb, \
             tc.tile_pool(name="sb", bufs=3) as sb, \
             tc.tile_pool(name="gp", bufs=2) as gp, \
             tc.tile_pool(name="ps", bufs=2, space="PSUM") as ps, \
             tc.tile_pool(name="dr", bufs=1, space="DRAM") as dr, \
             tc.tile_pool(name="tdr", bufs=2, space="DRAM") as tdr, \
             tc.tile_pool(name="adr", bufs=2, space="DRAM") as adr:

            WT = {}
            for name in list(wshapes.keys()) + ["sidx_ui", "sidx_iu"]:
                src_ap = I[name]
                dt = i16 if name.startswith("sidx") else f32
                t = wsb.tile(list(src_ap.shape), dt, name=f"w_{name}",
                             tag=f"w_{name}")
                nc.sync.dma_start(t[:], src_ap[:])
                WT[name] = t
            ident16 = wsb.tile([P, P], bf16, name="w_ident16", tag="w_ident16")
            nc.vector.tensor_copy(out=ident16[:], in_=WT["ident"][:])
            WT["ident16"] = ident16

            ag_u = dr.tile([ns, tblw], f32, name="ag_u", tag="ag_u")
            ag_i = dr.tile([ns, tblw], f32, name="ag_i", tag="ag_i")

            def new_a(side, d):
                return [adr.tile([P, d], f32, name=f"a_{side}_{b}",
                                 tag=f"a_{side}_{b}") for b in range(nb)]

            def new_ed(dirname):
                return [adr.tile([P, 4], f32, name=f"ed_{dirname}_{b}",
                                 tag=f"ed_{dirname}_{b}") for b in range(nb)]

            def new_table(side, l):
                return tdr.tile([cfg.tbl_rows, tblw], f32, addr_space="Shared",
                                name=f"table_{side}{l}", tag=f"table_{side}")

            def preproj(xname, wname, bname, a_tiles):
                for t in range(nb):
                    xt = sb.tile([P, P], f32, name="pp_x", tag="pp_x")
                    nc.sync.dma_start(xt[:], I[xname][:, t * P:(t + 1) * P])
                    pst = ps.tile([P, cfg.hid], f32, name="pp_ps", tag="ps_mm")
                    nc.tensor.matmul(out=pst[:], lhsT=xt[:], rhs=WT[wname][:],
                                     start=True, stop=True)
                    ot = sb.tile([P, cfg.hid], f32, name="pp_o", tag="pp_o")
                    nc.vector.tensor_add(out=ot[:], in0=pst[:], in1=WT[bname][:])
                    nc.scalar.activation(out=ot[:], in_=ot[:], func=AF.Relu)
                    nc.sync.dma_start(a_tiles[t][:], ot[:])

            def node_phase(a_tiles, din, wpbase, ag_dst, ed_tiles):
                nk = (din + P - 1) // P
                for t in range(nb):
                    at = sb.tile([P, din], f32, name="np_a", tag="np_a")
                    nc.sync.dma_start(at[:], a_tiles[t][:])
                    aT = sb.tile([P, nk * P], f32, name="np_aT", tag="np_aT")
                    for k in range(nk):
                        kw = min(P, din - k * P)
                        tp = ps.tile([P, P], f32, name="np_tp", tag="ps_tpf")
                        nc.tensor.transpose(out=tp[:kw, :],
                                            in_=at[:, k * P:k * P + kw],
                                            identity=WT["ident"][:])
                        nc.vector.tensor_copy(out=aT[:kw, k * P:(k + 1) * P],
                                              in_=tp[:kw, :])
                    hs = ps.tile([P, d1 + 8], f32, name="np_hs", tag="ps_mm")
                    for k in range(nk):
                        kw = min(P, din - k * P)
                        nc.tensor.matmul(out=hs[:],
                                         lhsT=aT[:kw, k * P:(k + 1) * P],
                                         rhs=WT[f"{wpbase}_{k}"][:],
                                         start=(k == 0), stop=(k == nk - 1))
                    agt = sb.tile([P, tblw], f32, name="np_ag", tag="np_ag")
                    nc.vector.tensor_copy(out=agt[:, 0:d1 + 4], in_=hs[:, 0:d1 + 4])
                    nc.vector.memset(agt[:, d1 + 4:tblw], 0.0)
                    nc.sync.dma_start(ag_dst[t * P:(t + 1) * P, :], agt[:])
                    edt = sb.tile([P, 4], f32, name="np_ed", tag="np_ed")
                    nc.vector.tensor_copy(out=edt[:], in_=hs[:, d1 + 4:d1 + 8])
                    nc.sync.dma_start(ed_tiles[t][:], edt[:])

            def allgather(ag_src, table):
                nc.gpsimd.collective_compute(
                    "AllGather", mybir.AluOpType.bypass, replica_groups=rg,
                    ins=[ag_src[:].opt()], outs=[table[:].opt()])

            def edge_phase(cpb, counts, table, sidx, dstf, ed_tiles,
                           bias_name, a_out, gtag):
                epb = cpb * P
                for b in range(nb):
                    g = gp.tile([P, cpb, tblw], f32, name=f"g_{gtag}",
                                tag=f"g_{gtag}")
                    nc.gpsimd.dma_gather(
                        g[:], table[:],
                        WT[sidx][:, b * (epb // 16):(b + 1) * (epb // 16)],
                        num_idxs=epb, num_idxs_reg=int(counts[b]),
                        elem_size=tblw, single_packet=False)
                    dstv = sb.tile([P, cpb], f32, name="eg_dst", tag="eg_dst")
                    nc.sync.dma_start(dstv[:], I[dstf][b, :, :])
                    edb = sb.tile([P, 4], f32, name="eg_edb", tag="eg_edb")
                    nc.sync.dma_start(edb[:], ed_tiles[b][:])
                    edb16 = sb.tile([P, 4], bf16, name="eg_edb16", tag="eg_edb16")
                    nc.vector.tensor_copy(out=edb16[:], in_=edb[:])
                    mask = sb.tile([P, cpb, P], bf16, name="eg_m", tag="eg_m")
                    nc.vector.tensor_tensor(
                        out=mask[:],
                        in0=dstv[:].to_broadcast([P, cpb, P]),
                        in1=WT["iota"][:].rearrange("p (o e) -> p o e", o=1)
                            .to_broadcast([P, cpb, P]),
                        op=AL.is_equal)
                    wv = sb.tile([P, cpb * 4], f32, name="eg_w", tag="eg_w")
                    nc.vector.tensor_copy(
                        out=wv[:].rearrange("p (c h) -> p c h", h=4),
                        in_=g[:, :, d1:d1 + 4])
                    edpe = ps.tile([P, cpb * 4], f32, name="eg_edpe",
                                   tag="ps_den")
                    for c in range(cpb):
                        tp = ps.tile([P, P], bf16, name="eg_tp", tag="ps_tp")
                        nc.tensor.transpose(out=tp[:], in_=mask[:, c, :],
                                            identity=WT["ident16"][:])
                        mT = sb.tile([P, P], bf16, name="eg_mT", tag="eg_mT")
                        nc.scalar.activation(out=mT[:], in_=tp[:], func=AF.Copy)
                        nc.tensor.matmul(out=edpe[:, c * 4:(c + 1) * 4],
                                         lhsT=mT[:], rhs=edb16[:],
                                         start=True, stop=True)
                    nc.vector.tensor_add(out=wv[:], in0=wv[:], in1=edpe[:])
                    tmp = sb.tile([P, cpb * 4], f32, name="eg_t", tag="eg_t")
                    nc.vector.tensor_scalar_mul(out=tmp[:], in0=wv[:],
                                                scalar1=float(cfg.neg))
                    nc.vector.tensor_tensor(out=wv[:], in0=wv[:], in1=tmp[:],
                                            op=AL.max)
                    nc.scalar.activation(out=wv[:], in_=wv[:], func=AF.Exp)
                    msg = gp.tile([P, cpb, d1 + 4], bf16, name=f"msg_{gtag}",
                                  tag=f"msg_{gtag}")
                    nc.vector.tensor_tensor(
                        out=msg[:, :, 0:d1].rearrange("p c (h f) -> p c h f", h=H),
                        in0=g[:, :, 0:d1].rearrange("p c (h f) -> p c h f", h=H),
                        in1=wv[:].rearrange("p (c h) -> p c h", h=4)
                            .to_broadcast([P, cpb, H, C]),
                        op=AL.mult)
                    nc.vector.tensor_copy(
                        out=msg[:, :, d1:d1 + 4],
                        in_=wv[:].rearrange("p (c h) -> p c h", h=4))
                    acc = ps.tile([P, d1 + 4], f32, name="eg_ps", tag="ps_mm")
                    for c in range(cpb):
                        nc.tensor.matmul(out=acc[:], lhsT=mask[:, c, :],
                                         rhs=msg[:, c, :],
                                         start=(c == 0), stop=(c == cpb - 1))
                    rec = sb.tile([P, 4], f32, name="eg_r", tag="eg_r")
                    nc.vector.tensor_scalar_add(out=rec[:], in0=acc[:, d1:d1 + 4],
                                                scalar1=1e-16)
                    nc.vector.reciprocal(out=rec[:], in_=rec[:])
                    ot = sb.tile([P, d1], f32, name="eg_o", tag="eg_o")
                    for h in range(H):
                        nc.vector.tensor_scalar_mul(
                            out=ot[:, h * C:(h + 1) * C],
                            in0=acc[:, h * C:(h + 1) * C],
                            scalar1=rec[:, h:h + 1])
                    nc.vector.tensor_add(out=ot[:], in0=ot[:],
                                         in1=WT[bias_name][:])
                    nc.scalar.activation(out=ot[:], in_=ot[:], func=AF.Relu)
                    nc.sync.dma_start(a_out[b][:], ot[:])

            def post_mlp(a_tiles, side, out_dst):
                for t in range(nb):
                    at = sb.tile([P, d1], f32, name="pm_a", tag="np_a")
                    nc.sync.dma_start(at[:], a_tiles[t][:])
                    aT = sb.tile([P, 2 * P], f32, name="pm_aT", tag="np_aT")
                    for k in range(2):
                        tp = ps.tile([P, P], f32, name="pm_tp", tag="ps_tpf")
                        nc.tensor.transpose(out=tp[:],
                                            in_=at[:, k * P:(k + 1) * P],
                                            identity=WT["ident"][:])
                        nc.vector.tensor_copy(out=aT[:, k * P:(k + 1) * P],
                                              in_=tp[:])
                    m1p = ps.tile([P, cfg.hid], f32, name="pm_ps1", tag="ps_mm")
                    for k in range(2):
                        nc.tensor.matmul(out=m1p[:],
                                         lhsT=aT[:, k * P:(k + 1) * P],
                                         rhs=WT[f"p0_{side}_{k}"][:],
                                         start=(k == 0), stop=(k == 1))
                    m1 = sb.tile([P, cfg.hid], f32, name="pm_m1", tag="pm_m1")
                    nc.vector.tensor_add(out=m1[:], in0=m1p[:],
                                         in1=WT[f"b0_{side}"][:])
                    nc.scalar.activation(out=m1[:], in_=m1[:], func=AF.Relu)
                    tp2 = ps.tile([P, P], f32, name="pm_tp2", tag="ps_tpf")
                    nc.tensor.transpose(out=tp2[:cfg.hid, :], in_=m1[:],
                                        identity=WT["ident"][:])
                    m1T = sb.tile([cfg.hid, P], f32, name="pm_m1T", tag="pm_m1T")
                    nc.vector.tensor_copy(out=m1T[:], in_=tp2[:cfg.hid, :])
                    op2 = ps.tile([P, cfg.hid], f32, name="pm_ps2", tag="ps_mm")
                    nc.tensor.matmul(out=op2[:], lhsT=m1T[:],
                                     rhs=WT[f"p1_{side}"][:],
                                     start=True, stop=True)
                    ob = sb.tile([P, cfg.hid], f32, name="pm_o", tag="pp_o")
                    nc.vector.tensor_add(out=ob[:], in0=op2[:],
                                         in1=WT[f"b1_{side}"][:])
                    nc.sync.dma_start(out_dst[t * P:(t + 1) * P, :], ob[:])

            UI = dict(cpb=cpb_ui, counts=cnt_ui, sidx="sidx_ui",
                      dstf="dstf_ui", gtag="ui")
            IU = dict(cpb=cpb_iu, counts=cnt_iu, sidx="sidx_iu",
                      dstf="dstf_iu", gtag="iu")

            a_u = new_a("u", cfg.hid)
            a_i = new_a("i", cfg.hid)
            preproj("xT_u", "w_pre_u", "b_pre_u", a_u)
            preproj("xT_i", "w_pre_i", "b_pre_i", a_i)

            ed_ui = new_ed("ui")
            ed_iu = new_ed("iu")
            tbl_u = new_table("u", 0)
            tbl_i = new_table("i", 0)
            node_phase(a_u, cfg.hid, "wp_u_0", ag_u, ed_iu)
            allgather(ag_u, tbl_u)
            node_phase(a_i, cfg.hid, "wp_i_0", ag_i, ed_ui)
            allgather(ag_i, tbl_i)

            # layer 0: ui then iu; table AGs for the next layer are emitted
            # right after the edge phase that produces their inputs, so the
            # collective overlaps the other direction's edge phase.
            a_i1 = new_a("i", d1)
            edge_phase(**UI, table=tbl_u, ed_tiles=ed_ui, bias_name="bias_ui_0",
                       a_out=a_i1)
            ed_ui1 = new_ed("ui")
            tbl_i1 = new_table("i", 1)
            node_phase(a_i1, d1, "wp_i_1", ag_i, ed_ui1)
            allgather(ag_i, tbl_i1)
            a_u1 = new_a("u", d1)
            edge_phase(**IU, table=tbl_i, ed_tiles=ed_iu, bias_name="bias_iu_0",
                       a_out=a_u1)
            ed_iu1 = new_ed("iu")
            tbl_u1 = new_table("u", 1)
            node_phase(a_u1, d1, "wp_u_1", ag_u, ed_iu1)
            allgather(ag_u, tbl_u1)

            # layer 1: iu first (table_i ready first)
            a_u2 = new_a("u", d1)
            edge_phase(**IU, table=tbl_i1, ed_tiles=ed_iu1,
                       bias_name="bias_iu_1", a_out=a_u2)
            ed_iu2 = new_ed("iu")
            tbl_u2 = new_table("u", 2)
            node_phase(a_u2, d1, "wp_u_2", ag_u, ed_iu2)
            allgather(ag_u, tbl_u2)
            a_i2 = new_a("i", d1)
            edge_phase(**UI, table=tbl_u1, ed_tiles=ed_ui1,
                       bias_name="bias_ui_1", a_out=a_i2)
            ed_ui2 = new_ed("ui")
            tbl_i2 = new_table("i", 2)
            node_phase(a_i2, d1, "wp_i_2", ag_i, ed_ui2)
            allgather(ag_i, tbl_i2)

            # layer 2: ui first (table_u ready first)
            a_i3 = new_a("i", d1)
            edge_phase(**UI, table=tbl_u2, ed_tiles=ed_ui2,
                       bias_name="bias_ui_2", a_out=a_i3)
            post_mlp(a_i3, "i", out_i)
            a_u3 = new_a("u", d1)
            edge_phase(**IU, table=tbl_i2, ed_tiles=ed_iu2,
                       bias_name="bias_iu_2", a_out=a_u3)
            post_mlp(a_u3, "u", out_u)

    nc.compile()
    return nc


def _prepare(x_user, x_item, edge_ui, edge_iu, params, cfg):
    cpb_ui, cnt_ui, sidx_ui, dstf_ui = _prep_dir(np.asarray(edge_ui), cfg)
    cpb_iu, cnt_iu, sidx_iu, dstf_iu = _prep_dir(np.asarray(edge_iu), cfg)
    weights = _prep_weights(params, cfg)
    xT_u = _node_slices(x_user, cfg)
    xT_i = _node_slices(x_item, cfg)
    in_maps = []
    for k in range(cfg.ncores):
        m = {
            "xT_u": xT_u[k], "xT_i": xT_i[k],
            "sidx_ui": sidx_ui[k],
            "dstf_ui": np.ascontiguousarray(dstf_ui[k]),
            "sidx_iu": sidx_iu[k],
            "dstf_iu": np.ascontiguousarray(dstf_iu[k]),
        }
        m.update(weights)
        in_maps.append(m)
    wshapes = {k: v.shape for k, v in weights.items()}
    return (cpb_ui, cpb_iu, cnt_ui, cnt_iu), wshapes, in_maps


def _install_ntff_hook():
    """Provide antenv.axon_hooks via ctypes when the image lacks it."""
    import types
    try:
        from antenv.axon_hooks import get_axon_ntff_profile_hook  # noqa: F401
        return
    except ImportError:
        pass
    try:
        from trn_agent_boot.trn_boot import _ntff_profile_via_ctypes
        hook = _ntff_profile_via_ctypes('/opt/axon/libaxon_pjrt.so')
    except Exception:
        return
    mod = types.ModuleType('antenv.axon_hooks')
    mod.get_axon_ntff_profile_hook = lambda: hook
    sys.modules['antenv.axon_hooks'] = mod


def _run(x_user, x_item, edge_ui, edge_iu, params, cfg=None, trace=False):
    from concourse import bass_utils
    if trace:
        _install_ntff_hook()
    cfg = cfg or Cfg()
    (cpb_ui, cpb_iu, cnt_ui, cnt_iu), wshapes, in_maps = _prepare(
        x_user, x_item, edge_ui, edge_iu, params, cfg)
    nc = _build(cfg, cpb_ui, cpb_iu, cnt_ui, cnt_iu, wshapes)
    res = bass_utils.run_bass_kernel_spmd(
        nc, in_maps, core_ids=list(range(cfg.ncores)), trace=trace)
    out_u = np.concatenate(
        [res.results[k]["out_u"][:cfg.nsr] for k in range(cfg.ncores)], 0)
    out_i = np.concatenate(
        [res.results[k]["out_i"][:cfg.nsr] for k in range(cfg.ncores)], 0)
    return (out_u, out_i), res


def kernel(x_user, x_item, edge_ui, edge_iu, params):
    (out_u, out_i), _ = _run(x_user, x_item, edge_ui, edge_iu, params)
    return out_u, out_i


# revision 17
# speedup vs baseline: 1.8096x; 1.1784x over previous
"""Bipartite 3-layer GAT (user<->item) on 8 Trainium2 NeuronCores.

Strategy:
- Destination-range sharding: core k owns user-nodes and item-nodes
  [k*2500, (k+1)*2500) as edge destinations. All edges whose dst falls in
  that range are processed by core k (sorted by dst on host).
- Node phase is sharded by node rows; each core computes its slice of
  h = a @ W (plus per-head attention projections es/ed), then an
  AllGather replicates the (h|es) table so every core can gather rows
  for arbitrary source nodes.
- Edge phase: per 128-dst block, batched dma_gather of source rows
  (h|es) and of per-dst ed rows, softmax weights w = exp(leaky(es+ed))
  (the segment-max cancels mathematically and scores are small, so it
  is skipped), messages scaled by w, and a one-hot matmul segment-sum
  accumulated in PSUM. Softmax denominators accumulate in the same
  PSUM tile via a second matmul against w.
- Host does index preprocessing only (shard/sort/pad); all FLOPs and
  data movement run on device.
"""
import sys

for _p in ("/opt/trn_rl_repo",):
    if _p not in sys.path:
        sys.path.insert(0, _p)

import numpy as np

P = 128


class Cfg:
    def __init__(self, n=20000, e=320000, f_in=128, hid=64, heads=4, nl=3,
                 neg=0.2, ncores=8):
        self.n, self.e, self.f_in, self.hid = n, e, f_in, hid
        self.heads, self.nl, self.neg, self.ncores = heads, nl, neg, ncores
        self.nsr = n // ncores                    # raw nodes per core
        self.nb = (self.nsr + P - 1) // P         # dst blocks per core
        self.ns = self.nb * P                     # padded nodes per core
        self.d1 = heads * hid                     # 256
        self.tblw = self.d1 + 128                 # bf16 [hs|es_hi|es_lo|pad], 256B-mult rows
        self.tbl_rows = ncores * self.ns


def _wrap_idx(ix):
    """[T] int -> [128, T//16] int16 (16-partition wrap, replicated 8x)."""
    assert ix.shape[0] % 16 == 0
    w = ix.reshape(-1, 16).T.astype(np.int16)
    return np.ascontiguousarray(np.tile(w, (8, 1)))


def _prep_dir(edge, cfg):
    """Shard+sort one edge direction by dst range. Returns (cpb, counts[nb],
    src_w [NC,128,*], masks [NC,nb,128,cpb,128], maskTs same)."""
    src = np.asarray(edge[0]).astype(np.int64)
    dst = np.asarray(edge[1]).astype(np.int64)
    percore = []
    blockcnt = np.zeros((cfg.ncores, cfg.nb), np.int64)
    for k in range(cfg.ncores):
        m = (dst >= k * cfg.nsr) & (dst < (k + 1) * cfg.nsr)
        s = src[m]
        d = dst[m] - k * cfg.nsr
        o = np.argsort(d, kind="stable")
        s, d = s[o], d[o]
        blockcnt[k] = np.bincount(d // P, minlength=cfg.nb)
        percore.append((s, d))
    cpb = max(1, (int(blockcnt.max()) + P - 1) // P)
    counts = blockcnt.max(axis=0)
    counts = np.minimum(((counts + 15) // 16) * 16, cpb * P)
    counts[0:2] = cpb * P  # first two blocks fully sanitize the SBUF slots
    epc = cfg.nb * cpb * P
    pad_row = cfg.ns - 1
    src_rows = np.full((cfg.ncores, epc), -1, np.int64)
    dstf = np.full((cfg.ncores, cfg.nb, P, cpb), -1, np.int64)
    for k, (s, d) in enumerate(percore):
        starts = np.concatenate([[0], np.cumsum(blockcnt[k])])
        srow = (s // cfg.nsr) * cfg.ns + (s % cfg.nsr)
        for b in range(cfg.nb):
            cnt = int(blockcnt[k][b])
            base = b * cpb * P
            sl = slice(int(starts[b]), int(starts[b]) + cnt)
            src_rows[k, base:base + cnt] = srow[sl]
            src_rows[k, base + cnt:base + int(counts[b])] = pad_row
            j = np.arange(cnt)
            dstf[k, b, j % P, j // P] = d[sl] - b * P
    src_w = np.stack([_wrap_idx(src_rows[k]) for k in range(cfg.ncores)])
    # dense one-hot masks, both orientations, bf16
    try:
        from ml_dtypes import bfloat16 as _bf
    except ImportError:
        import jax.numpy as _jnp
        _bf = _jnp.bfloat16
    eye = np.arange(P)
    mask = (dstf[:, :, :, :, None] == eye).astype(_bf)     # [NC,nb,128e,cpb,128d]
    maskT = np.swapaxes(mask, 2, 4).copy()                 # [NC,nb,128d,cpb,128e]
    return cpb, counts, src_w, np.ascontiguousarray(mask), maskT


def _f32(x):
    return np.ascontiguousarray(np.asarray(x), dtype=np.float32)


def _bcast(v):
    v = _f32(v)
    return np.ascontiguousarray(np.broadcast_to(v[None, :], (P, v.shape[0])))


def _ktiles(w):
    """[din, m] -> list of [<=128, m] row tiles."""
    w = _f32(w)
    return [np.ascontiguousarray(w[k * P:(k + 1) * P])
            for k in range((w.shape[0] + P - 1) // P)]


def _prep_weights(params, cfg):
    H, C = cfg.heads, cfg.hid
    out = {}
    out["w_pre_u"] = _f32(params["pre_u"][0])
    out["b_pre_u"] = _bcast(params["pre_u"][1])
    out["w_pre_i"] = _f32(params["pre_i"][0])
    out["b_pre_i"] = _bcast(params["pre_i"][1])

    def vproj(W, a):
        W = _f32(W); a = _f32(a)
        return np.stack([W[:, h * C:(h + 1) * C] @ a[h] for h in range(H)], 1)

    for l, lp in enumerate(params["layers"]):
        Wui, asui, adui, bui = lp["ui"]
        Wiu, asiu, adiu, biu = lp["iu"]
        wp_u = np.concatenate([_f32(Wui), vproj(Wui, asui), vproj(Wiu, adiu)], 1)
        wp_i = np.concatenate([_f32(Wiu), vproj(Wiu, asiu), vproj(Wui, adui)], 1)
        for k, t in enumerate(_ktiles(wp_u)):
            out[f"wp_u_{l}_{k}"] = t
        for k, t in enumerate(_ktiles(wp_i)):
            out[f"wp_i_{l}_{k}"] = t
        out[f"bias_ui_{l}"] = _bcast(bui)
        out[f"bias_iu_{l}"] = _bcast(biu)
    for side in ("u", "i"):
        p0, b0, p1, b1 = params[f"post_{side}"]
        for k, t in enumerate(_ktiles(p0)):
            out[f"p0_{side}_{k}"] = t
        out[f"b0_{side}"] = _bcast(b0)
        out[f"p1_{side}"] = _f32(p1)
        out[f"b1_{side}"] = _bcast(b1)
    out["iota"] = np.ascontiguousarray(
        np.broadcast_to(np.arange(P, dtype=np.float32)[None, :], (P, P)))
    out["ident"] = np.eye(P, dtype=np.float32)
    return out


def _node_slices(x, cfg):
    """Full [n, f] -> per-core transposed padded [f, ns]."""
    x = _f32(x)
    outs = []
    for k in range(cfg.ncores):
        sl = x[k * cfg.nsr:(k + 1) * cfg.nsr]
        pad = np.zeros((cfg.ns, x.shape[1]), np.float32)
        pad[:sl.shape[0]] = sl
        outs.append(np.ascontiguousarray(pad.T))
    return outs


def _build(cfg, cpb_ui, cpb_iu, cnt_ui, cnt_iu, wshapes):
    import concourse.bass as bass
    import concourse.bacc as bacc
    import concourse.tile as tile
    import concourse.mybir as mybir

    f32 = mybir.dt.float32
    bf16 = mybir.dt.bfloat16
    i16 = mybir.dt.int16
    AL = mybir.AluOpType
    AF = mybir.ActivationFunctionType
    SUB = getattr(AL, "subtract", None)
    nb, ns, d1, tblw, H, C = cfg.nb, cfg.ns, cfg.d1, cfg.tblw, cfg.heads, cfg.hid

    nc = bacc.Bacc("TRN2", target_bir_lowering=False, debug=False,
                   num_devices=cfg.ncores)
    I = {}

    def inp(name, shape, dt=f32):
        I[name] = nc.dram_tensor(name, list(shape), dt, kind="ExternalInput").ap()

    inp("xT_u", [cfg.f_in, ns])
    inp("xT_i", [cfg.f_in, ns])
    epc_ui, epc_iu = nb * cpb_ui * P, nb * cpb_iu * P
    inp("sidx_ui", [P, epc_ui // 16], i16)
    inp("mk_ui", [nb, P, cpb_ui, P], bf16)
    inp("mkT_ui", [nb, P, cpb_ui, P], bf16)
    inp("sidx_iu", [P, epc_iu // 16], i16)
    inp("mk_iu", [nb, P, cpb_iu, P], bf16)
    inp("mkT_iu", [nb, P, cpb_iu, P], bf16)
    for name, arr_shape in wshapes.items():
        inp(name, list(arr_shape))

    out_u = nc.dram_tensor("out_u", [ns, cfg.hid], f32, kind="ExternalOutput").ap()
    out_i = nc.dram_tensor("out_i", [ns, cfg.hid], f32, kind="ExternalOutput").ap()

    rg = [list(range(cfg.ncores))]

    with tile.TileContext(nc) as tc:
        with tc.tile_pool(name="wsb", bufs=1) as wsb, \
             tc.tile_pool(name="sb", bufs=3) as sb, \
             tc.tile_pool(name="gp", bufs=2) as gp, \
             tc.tile_pool(name="ps", bufs=2, space="PSUM") as ps, \
             tc.tile_pool(name="dr", bufs=1, space="DRAM") as dr, \
             tc.tile_pool(name="tdr", bufs=2, space="DRAM") as tdr, \
             tc.tile_pool(name="adr", bufs=2, space="DRAM") as adr:

            WT = {}
            for name in list(wshapes.keys()) + ["sidx_ui", "sidx_iu"]:
                src_ap = I[name]
                dt = i16 if name.startswith("sidx") else f32
                t = wsb.tile(list(src_ap.shape), dt, name=f"w_{name}",
                             tag=f"w_{name}")
                nc.sync.dma_start(t[:], src_ap[:])
                WT[name] = t
            ag_u = dr.tile([ns, tblw], bf16, name="ag_u", tag="ag_u")
            ag_i = dr.tile([ns, tblw], bf16, name="ag_i", tag="ag_i")

            def new_a(side, d):
                return [adr.tile([P, d], f32, name=f"a_{side}_{b}",
                                 tag=f"a_{side}_{b}") for b in range(nb)]

            def new_ed(dirname):
                return [adr.tile([P, 4], f32, name=f"ed_{dirname}_{b}",
                                 tag=f"ed_{dirname}_{b}") for b in range(nb)]

            def new_table(side, l):
                return tdr.tile([cfg.tbl_rows, tblw], bf16, addr_space="Shared",
                                name=f"table_{side}{l}", tag=f"table_{side}")

            def preproj(xname, wname, bname, a_tiles):
                for t in range(nb):
                    xt = sb.tile([P, P], f32, name="pp_x", tag="pp_x")
                    nc.sync.dma_start(xt[:], I[xname][:, t * P:(t + 1) * P])
                    pst = ps.tile([P, cfg.hid], f32, name="pp_ps", tag="ps_mm")
                    nc.tensor.matmul(out=pst[:], lhsT=xt[:], rhs=WT[wname][:],
                                     start=True, stop=True)
                    ot = sb.tile([P, cfg.hid], f32, name="pp_o", tag="pp_o")
                    nc.vector.tensor_add(out=ot[:], in0=pst[:], in1=WT[bname][:])
                    nc.scalar.activation(out=ot[:], in_=ot[:], func=AF.Relu)
                    nc.sync.dma_start(a_tiles[t][:], ot[:])

            def node_phase(a_tiles, din, wpbase, ag_dst, ed_tiles):
                nk = (din + P - 1) // P
                for t in range(nb):
                    at = sb.tile([P, din], f32, name="np_a", tag="np_a")
                    nc.sync.dma_start(at[:], a_tiles[t][:])
                    aT = sb.tile([P, nk * P], f32, name="np_aT", tag="np_aT")
                    for k in range(nk):
                        kw = min(P, din - k * P)
                        tp = ps.tile([P, P], f32, name="np_tp", tag="ps_tpf")
                        nc.tensor.transpose(out=tp[:kw, :],
                                            in_=at[:, k * P:k * P + kw],
                                            identity=WT["ident"][:])
                        nc.vector.tensor_copy(out=aT[:kw, k * P:(k + 1) * P],
                                              in_=tp[:kw, :])
                    hs = ps.tile([P, d1 + 8], f32, name="np_hs", tag="ps_mm")
                    for k in range(nk):
                        kw = min(P, din - k * P)
                        nc.tensor.matmul(out=hs[:],
                                         lhsT=aT[:kw, k * P:(k + 1) * P],
                                         rhs=WT[f"{wpbase}_{k}"][:],
                                         start=(k == 0), stop=(k == nk - 1))
                    agt = sb.tile([P, tblw], bf16, name="np_ag", tag="np_ag")
                    nc.vector.tensor_copy(out=agt[:, 0:d1 + 4], in_=hs[:, 0:d1 + 4])
                    eshi = sb.tile([P, 4], f32, name="np_eshi", tag="np_eshi")
                    nc.vector.tensor_copy(out=eshi[:], in_=agt[:, d1:d1 + 4])
                    eslo = sb.tile([P, 4], f32, name="np_eslo", tag="np_eslo")
                    if SUB is not None:
                        nc.vector.tensor_tensor(out=eslo[:], in0=hs[:, d1:d1 + 4],
                                                in1=eshi[:], op=SUB)
                    else:
                        nc.vector.tensor_scalar_mul(out=eshi[:], in0=eshi[:],
                                                    scalar1=-1.0)
                        nc.vector.tensor_add(out=eslo[:], in0=hs[:, d1:d1 + 4],
                                             in1=eshi[:])
                    nc.vector.tensor_copy(out=agt[:, d1 + 4:d1 + 8], in_=eslo[:])
                    nc.vector.memset(agt[:, d1 + 8:tblw], 0.0)
                    nc.sync.dma_start(ag_dst[t * P:(t + 1) * P, :], agt[:])
                    edt = sb.tile([P, 4], f32, name="np_ed", tag="np_ed")
                    nc.vector.tensor_copy(out=edt[:], in_=hs[:, d1 + 4:d1 + 8])
                    nc.sync.dma_start(ed_tiles[t][:], edt[:])

            def allgather(ag_src, table):
                nc.gpsimd.collective_compute(
                    "AllGather", mybir.AluOpType.bypass, replica_groups=rg,
                    ins=[ag_src[:].opt()], outs=[table[:].opt()])

            def edge_phase(cpb, counts, table, sidx, mk, mkT, ed_tiles,
                           bias_name, a_out, gtag):
                epb = cpb * P
                for b in range(nb):
                    cb = cpb if b < 2 else max(1, (int(counts[b]) + P - 1) // P)
                    g = gp.tile([P, cpb, tblw], bf16, name=f"g_{gtag}",
                                tag=f"g_{gtag}")
                    nc.gpsimd.dma_gather(
                        g[:, 0:cb, :], table[:],
                        WT[sidx][:, b * (epb // 16):
                                 b * (epb // 16) + (cb * P) // 16],
                        num_idxs=cb * P, num_idxs_reg=int(counts[b]),
                        elem_size=tblw, single_packet=False)
                    mka = sb.tile([P, cpb, P], bf16, name="eg_mk", tag="eg_mk")
                    nc.sync.dma_start(mka[:, 0:cb, :], I[mk][b, :, 0:cb, :])
                    mkTa = sb.tile([P, cpb, P], bf16, name="eg_mkT", tag="eg_mkT")
                    nc.sync.dma_start(mkTa[:, 0:cb, :], I[mkT][b, :, 0:cb, :])
                    edb = sb.tile([P, 4], f32, name="eg_edb", tag="eg_edb")
                    nc.sync.dma_start(edb[:], ed_tiles[b][:])
                    edb16 = sb.tile([P, 4], bf16, name="eg_edb16", tag="eg_edb16")
                    nc.vector.tensor_copy(out=edb16[:], in_=edb[:])
                    wv = sb.tile([P, cpb * 4], f32, name="eg_w", tag="eg_w")
                    nc.vector.tensor_copy(
                        out=wv[:, 0:cb * 4].rearrange("p (c h) -> p c h", h=4),
                        in_=g[:, 0:cb, d1:d1 + 4])
                    tmp = sb.tile([P, cpb * 4], f32, name="eg_t", tag="eg_t")
                    nc.vector.tensor_copy(
                        out=tmp[:, 0:cb * 4].rearrange("p (c h) -> p c h", h=4),
                        in_=g[:, 0:cb, d1 + 4:d1 + 8])
                    nc.vector.tensor_add(out=wv[:, 0:cb * 4],
                                         in0=wv[:, 0:cb * 4],
                                         in1=tmp[:, 0:cb * 4])
                    edpe = ps.tile([P, cpb * 4], f32, name="eg_edpe",
                                   tag="ps_den")
                    for c in range(cb):
                        nc.tensor.matmul(out=edpe[:, c * 4:(c + 1) * 4],
                                         lhsT=mkTa[:, c, :], rhs=edb16[:],
                                         start=True, stop=True)
                    nc.vector.tensor_add(out=wv[:, 0:cb * 4],
                                         in0=wv[:, 0:cb * 4],
                                         in1=edpe[:, 0:cb * 4])
                    nc.vector.tensor_scalar_mul(out=tmp[:, 0:cb * 4],
                                                in0=wv[:, 0:cb * 4],
                                                scalar1=float(cfg.neg))
                    nc.vector.tensor_tensor(out=wv[:, 0:cb * 4],
                                            in0=wv[:, 0:cb * 4],
                                            in1=tmp[:, 0:cb * 4], op=AL.max)
                    nc.scalar.activation(out=wv[:, 0:cb * 4],
                                         in_=wv[:, 0:cb * 4], func=AF.Exp)
                    wv16 = sb.tile([P, cpb * 4], bf16, name="eg_w16",
                                   tag="eg_w16")
                    nc.vector.tensor_copy(out=wv16[:, 0:cb * 4],
                                          in_=wv[:, 0:cb * 4])
                    msg = gp.tile([P, cpb, d1 + 4], bf16, name=f"msg_{gtag}",
                                  tag=f"msg_{gtag}")
                    nc.vector.tensor_tensor(
                        out=msg[:, 0:cb, 0:d1]
                            .rearrange("p c (h f) -> p c h f", h=H),
                        in0=g[:, 0:cb, 0:d1]
                            .rearrange("p c (h f) -> p c h f", h=H),
                        in1=wv16[:, 0:cb * 4].rearrange("p (c h) -> p c h", h=4)
                            .to_broadcast([P, cb, H, C]),
                        op=AL.mult)
                    nc.vector.tensor_copy(
                        out=msg[:, 0:cb, d1:d1 + 4],
                        in_=wv16[:, 0:cb * 4].rearrange("p (c h) -> p c h", h=4))
                    acc = ps.tile([P, d1 + 4], f32, name="eg_ps", tag="ps_mm")
                    for c in range(cb):
                        nc.tensor.matmul(out=acc[:], lhsT=mka[:, c, :],
                                         rhs=msg[:, c, :],
                                         start=(c == 0), stop=(c == cb - 1))
                    rec = sb.tile([P, 4], f32, name="eg_r", tag="eg_r")
                    nc.vector.tensor_scalar_add(out=rec[:], in0=acc[:, d1:d1 + 4],
                                                scalar1=1e-16)
                    nc.vector.reciprocal(out=rec[:], in_=rec[:])
                    ot = sb.tile([P, d1], f32, name="eg_o", tag="eg_o")
                    for h in range(H):
                        nc.vector.tensor_scalar_mul(
                            out=ot[:, h * C:(h + 1) * C],
                            in0=acc[:, h * C:(h + 1) * C],
                            scalar1=rec[:, h:h + 1])
                    nc.vector.tensor_add(out=ot[:], in0=ot[:],
                                         in1=WT[bias_name][:])
                    nc.scalar.activation(out=ot[:], in_=ot[:], func=AF.Relu)
                    nc.sync.dma_start(a_out[b][:], ot[:])

            def post_mlp(a_tiles, side, out_dst):
                for t in range(nb):
                    at = sb.tile([P, d1], f32, name="pm_a", tag="np_a")
                    nc.sync.dma_start(at[:], a_tiles[t][:])
                    aT = sb.tile([P, 2 * P], f32, name="pm_aT", tag="np_aT")
                    for k in range(2):
                        tp = ps.tile([P, P], f32, name="pm_tp", tag="ps_tpf")
                        nc.tensor.transpose(out=tp[:],
                                            in_=at[:, k * P:(k + 1) * P],
                                            identity=WT["ident"][:])
                        nc.vector.tensor_copy(out=aT[:, k * P:(k + 1) * P],
                                              in_=tp[:])
                    m1p = ps.tile([P, cfg.hid], f32, name="pm_ps1", tag="ps_mm")
                    for k in range(2):
                        nc.tensor.matmul(out=m1p[:],
                                         lhsT=aT[:, k * P:(k + 1) * P],
                                         rhs=WT[f"p0_{side}_{k}"][:],
                                         start=(k == 0), stop=(k == 1))
                    m1 = sb.tile([P, cfg.hid], f32, name="pm_m1", tag="pm_m1")
                    nc.vector.tensor_add(out=m1[:], in0=m1p[:],
                                         in1=WT[f"b0_{side}"][:])
                    nc.scalar.activation(out=m1[:], in_=m1[:], func=AF.Relu)
                    tp2 = ps.tile([P, P], f32, name="pm_tp2", tag="ps_tpf")
                    nc.tensor.transpose(out=tp2[:cfg.hid, :], in_=m1[:],
                                        identity=WT["ident"][:])
                    m1T = sb.tile([cfg.hid, P], f32, name="pm_m1T", tag="pm_m1T")
                    nc.vector.tensor_copy(out=m1T[:], in_=tp2[:cfg.hid, :])
                    op2 = ps.tile([P, cfg.hid], f32, name="pm_ps2", tag="ps_mm")
                    nc.tensor.matmul(out=op2[:], lhsT=m1T[:],
                                     rhs=WT[f"p1_{side}"][:],
                                     start=True, stop=True)
                    ob = sb.tile([P, cfg.hid], f32, name="pm_o", tag="pp_o")
                    nc.vector.tensor_add(out=ob[:], in0=op2[:],
                                         in1=WT[f"b1_{side}"][:])
                    nc.sync.dma_start(out_dst[t * P:(t + 1) * P, :], ob[:])

            UI = dict(cpb=cpb_ui, counts=cnt_ui, sidx="sidx_ui",
                      mk="mk_ui", mkT="mkT_ui", gtag="ui")
            IU = dict(cpb=cpb_iu, counts=cnt_iu, sidx="sidx_iu",
                      mk="mk_iu", mkT="mkT_iu", gtag="iu")

            a_u = new_a("u", cfg.hid)
            a_i = new_a("i", cfg.hid)
            preproj("xT_u", "w_pre_u", "b_pre_u", a_u)
            preproj("xT_i", "w_pre_i", "b_pre_i", a_i)

            ed_ui = new_ed("ui")
            ed_iu = new_ed("iu")
            tbl_u = new_table("u", 0)
            tbl_i = new_table("i", 0)
            node_phase(a_u, cfg.hid, "wp_u_0", ag_u, ed_iu)
            allgather(ag_u, tbl_u)
            node_phase(a_i, cfg.hid, "wp_i_0", ag_i, ed_ui)
            allgather(ag_i, tbl_i)

            # layer 0: ui then iu; table AGs for the next layer are emitted
            # right after the edge phase that produces their inputs, so the
            # collective overlaps the other direction's edge phase.
            a_i1 = new_a("i", d1)
            edge_phase(**UI, table=tbl_u, ed_tiles=ed_ui, bias_name="bias_ui_0",
                       a_out=a_i1)
            ed_ui1 = new_ed("ui")
            tbl_i1 = new_table("i", 1)
            node_phase(a_i1, d1, "wp_i_1", ag_i, ed_ui1)
            allgather(ag_i, tbl_i1)
            a_u1 = new_a("u", d1)
            edge_phase(**IU, table=tbl_i, ed_tiles=ed_iu, bias_name="bias_iu_0",
                       a_out=a_u1)
            ed_iu1 = new_ed("iu")
            tbl_u1 = new_table("u", 1)
            node_phase(a_u1, d1, "wp_u_1", ag_u, ed_iu1)
            allgather(ag_u, tbl_u1)

            # layer 1: iu first (table_i ready first)
            a_u2 = new_a("u", d1)
            edge_phase(**IU, table=tbl_i1, ed_tiles=ed_iu1,
                       bias_name="bias_iu_1", a_out=a_u2)
            ed_iu2 = new_ed("iu")
            tbl_u2 = new_table("u", 2)
            node_phase(a_u2, d1, "wp_u_2", ag_u, ed_iu2)
            allgather(ag_u, tbl_u2)
            a_i2 = new_a("i", d1)
            edge_phase(**UI, table=tbl_u1, ed_tiles=ed_ui1,
                       bias_name="bias_ui_1", a_out=a_i2)
            ed_ui2 = new_ed("ui")
            tbl_i2 = new_table("i", 2)
            node_phase(a_i2, d1, "wp_i_2", ag_i, ed_ui2)
            allgather(ag_i, tbl_i2)

            # layer 2: ui first (table_u ready first)
            a_i3 = new_a("i", d1)
            edge_phase(**UI, table=tbl_u2, ed_tiles=ed_ui2,
                       bias_name="bias_ui_2", a_out=a_i3)
            post_mlp(a_i3, "i", out_i)
            a_u3 = new_a("u", d1)
            edge_phase(**IU, table=tbl_i2, ed_tiles=ed_iu2,
                       bias_name="bias_iu_2", a_out=a_u3)
            post_mlp(a_u3, "u", out_u)

    nc.compile()
    return nc


def _prepare(x_user, x_item, edge_ui, edge_iu, params, cfg):
    cpb_ui, cnt_ui, sidx_ui, mk_ui, mkT_ui = _prep_dir(np.asarray(edge_ui), cfg)
    cpb_iu, cnt_iu, sidx_iu, mk_iu, mkT_iu = _prep_dir(np.asarray(edge_iu), cfg)
    weights = _prep_weights(params, cfg)
    xT_u = _node_slices(x_user, cfg)
    xT_i = _node_slices(x_item, cfg)
    in_maps = []
    for k in range(cfg.ncores):
        m = {
            "xT_u": xT_u[k], "xT_i": xT_i[k],
            "sidx_ui": sidx_ui[k], "mk_ui": mk_ui[k], "mkT_ui": mkT_ui[k],
            "sidx_iu": sidx_iu[k], "mk_iu": mk_iu[k], "mkT_iu": mkT_iu[k],
        }
        m.update(weights)
        in_maps.append(m)
    wshapes = {k: v.shape for k, v in weights.items()}
    return (cpb_ui, cpb_iu, cnt_ui, cnt_iu), wshapes, in_maps


def _install_ntff_hook():
    """Provide antenv.axon_hooks via ctypes when the image lacks it."""
    import types
    try:
        from antenv.axon_hooks import get_axon_ntff_profile_hook  # noqa: F401
        return
    except ImportError:
        pass
    try:
        from trn_agent_boot.trn_boot import _ntff_profile_via_ctypes
        hook = _ntff_profile_via_ctypes('/opt/axon/libaxon_pjrt.so')
    except Exception:
        return
    mod = types.ModuleType('antenv.axon_hooks')
    mod.get_axon_ntff_profile_hook = lambda: hook
    sys.modules['antenv.axon_hooks'] = mod


def _run(x_user, x_item, edge_ui, edge_iu, params, cfg=None, trace=False):
    from concourse import bass_utils
    if trace:
        _install_ntff_hook()
    cfg = cfg or Cfg()
    (cpb_ui, cpb_iu, cnt_ui, cnt_iu), wshapes, in_maps = _prepare(
        x_user, x_item, edge_ui, edge_iu, params, cfg)
    nc = _build(cfg, cpb_ui, cpb_iu, cnt_ui, cnt_iu, wshapes)
    res = bass_utils.run_bass_kernel_spmd(
        nc, in_maps, core_ids=list(range(cfg.ncores)), trace=trace)
    out_u = np.concatenate(
        [res.results[k]["out_u"][:cfg.nsr] for k in range(cfg.ncores)], 0)
    out_i = np.concatenate(
        [res.results[k]["out_i"][:cfg.nsr] for k in range(cfg.ncores)], 0)
    return (out_u, out_i), res


def kernel(x_user, x_item, edge_ui, edge_iu, params):
    (out_u, out_i), _ = _run(x_user, x_item, edge_ui, edge_iu, params)
    return out_u, out_i
